# revision 17
# baseline (speedup 1.0000x reference)
"""Trainium2 Bass kernel for grayscale+Canny+1x1-conv (nn_BFA_3015067042007).

Data-parallel over batch: 16 images -> 8 cores x 2 images.

v2 design (vs v1 baseline at ~470us):
  - Whole-image tiles [128, 4, 514]: the 4 row-strips of an image are packed
    along the free dim, so every elementwise Canny op covers the full image
    in ONE instruction (amortizes the ~400ns/instruction fixed overhead 4x).
    Stencil matmuls still run per strip on free-dim slices of the big tiles.
  - Vertical N/S neighbor shifts for NMS via SBUF->SBUF partition-shifted
    DMA instead of PE matmuls + PSUM evictions.
  - Fused 2-pass fp16 conv: rhs tile per strip is [128, 8, 512] with
    pseudo-channels [Rh Gh Bh E | Rl Gl Bl E] (hi/lo fp16 split of x in K).
    Pass A lhsT carries Wh on all rgb rows, (Wh_e, Wl_e) on the two edge
    rows; pass B carries Wl on rgb rows. Sum = (Wh+Wl)(xh+xl) + W_e*e with
    no dropped cross terms -> full fp32-level accuracy with 2 matmul passes
    (64 MMs/strip vs 96 for the 3-pass hi/lo scheme).
  - x is read from HBM once: conv inputs come from on-chip fp16 hi/lo
    copies of the P1 rgb tiles, folded into the conv layout by SBUF->SBUF
    DMAs (partition<->free exchange).
  - fp16 output (rel err ~5e-4 << 2e-2 tolerance): halves the dominant
    64MB/core output write; host upcasts to f32.
  - PSUM conv tiles are [128,1024] (2 banks): one bias+relu+fp16 eviction
    per 2 windows, spread across ACT/DVE/GPSIMD.
"""

import numpy as np

B_FULL = 16
N_CORES = 8
B_LOC = B_FULL // N_CORES
H = 512
W_IMG = 512
NS = 4  # row strips of 128

MAGIC_A = 8388607.5
MAGIC_B = 8388608.0
TG22 = 0.4142135623730951
TG67 = 2.414213562373095

# shift-matrix stack indices
I_T_TOP, I_T_MID, I_T_BOT = 0, 1, 2
I_D_TOP, I_D_MID, I_D_BOT = 3, 4, 5
I_V, I_H_TOP, I_H_BOT, I_H_TOP_D = 6, 7, 8, 9
N_MATS = 10


def build_shift_mats():
    m = np.zeros((N_MATS, 128, 128), np.float16)
    i = np.arange(128)
    # vertical (1,2,1) smooth: out[p] = in[p-1] + 2 in[p] + in[p+1]
    for t in (I_T_TOP, I_T_MID, I_T_BOT):
        m[t][i, i] = 2.0
        m[t][i[:-1], i[1:]] = 1.0
        m[t][i[1:], i[:-1]] = 1.0
    m[I_T_TOP][0, 0] = 3.0      # replicate pad at image top
    m[I_T_BOT][127, 127] = 3.0  # replicate pad at image bottom
    # vertical diff: out[p] = in[p+1] - in[p-1]
    for t in (I_D_TOP, I_D_MID, I_D_BOT):
        m[t][i[1:], i[:-1]] = 1.0
        m[t][i[:-1], i[1:]] = -1.0
    m[I_D_TOP][0, 0] = -1.0       # out[0] = in[1] - in[0]
    m[I_D_BOT][127, 127] = 1.0    # out[127] = in[127] - in[126]
    m[I_V][i, i] = 1.0            # vertical (1,1,1) sum
    m[I_V][i[:-1], i[1:]] = 1.0
    m[I_V][i[1:], i[:-1]] = 1.0
    m[I_H_TOP][127, 0] = 1.0      # prev strip row 127 -> out row 0
    m[I_H_BOT][0, 127] = 1.0      # next strip row 0 -> out row 127
    m[I_H_TOP_D][127, 0] = -1.0   # diff halo: -in_prev[127]
    return m


def build_conv_banks(W, b):
    """Two fp16 lhsT banks [128, 4, 128] for the fused 2-pass conv.

    rhs tile partition p: hi rgb 16c+rr (c<3), hi edge 48+rr, lo rgb
    64+16c+rr, lo edge 112+rr; rr = row within the 16-row window.
    psum m = 16*oi + rr, och = 8*og + oi.
    Pass A: Wh on all rgb rows; edge rows carry (Wh_e, Wl_e).
    Pass B: Wl on all rgb rows; edge rows 0."""
    Wc = W.astype(np.float32).copy()
    Wc[:, 3] = Wc[:, 3] * np.float32(255.0)
    Wh = Wc.astype(np.float16)
    Wl = (Wc - Wh.astype(np.float32)).astype(np.float16)
    convA = np.zeros((128, 4, 128), np.float16)
    convB = np.zeros((128, 4, 128), np.float16)
    rr = np.arange(16)
    for og in range(4):
        for oi in range(8):
            oc = 8 * og + oi
            m = 16 * oi + rr
            for c in range(3):
                convA[16 * c + rr, og, m] = Wh[oc, c]
                convA[64 + 16 * c + rr, og, m] = Wh[oc, c]
                convB[16 * c + rr, og, m] = Wl[oc, c]
                convB[64 + 16 * c + rr, og, m] = Wl[oc, c]
            convA[48 + rr, og, m] = Wh[oc, 3]
            convA[112 + rr, og, m] = Wl[oc, 3]
    p = np.arange(128)
    brep = np.stack([b[8 * og + p // 16] for og in range(4)], axis=1)
    return convA, convB, brep.astype(np.float32)


_PROG_CACHE = {}


def build_program():
    import concourse.bacc as bacc
    import concourse.tile as tile
    import concourse.mybir as mybir
    from concourse.mybir import AluOpType as op, ActivationFunctionType as act
    from contextlib import ExitStack

    f32 = mybir.dt.float32
    f16 = mybir.dt.float16
    u8 = mybir.dt.uint8

    nc = bacc.Bacc("TRN2", target_bir_lowering=False, debug=False)
    x_d = nc.dram_tensor("x", [B_LOC, 3, H, W_IMG], f32, kind="ExternalInput").ap()
    mats_d = nc.dram_tensor("mats", [N_MATS, 128, 128], f16, kind="ExternalInput").ap()
    convA_d = nc.dram_tensor("convA", [128, 512], f16, kind="ExternalInput").ap()
    convB_d = nc.dram_tensor("convB", [128, 512], f16, kind="ExternalInput").ap()
    brep_d = nc.dram_tensor("brep", [128, 4], f32, kind="ExternalInput").ap()
    out_d = nc.dram_tensor("out", [B_LOC, 32, H, W_IMG], f16, kind="ExternalOutput").ap()
    e_d = nc.dram_tensor("escratch", [B_LOC, H, W_IMG], f16, kind="Internal").ap()

    with tile.TileContext(nc) as tc:
        with ExitStack() as ctx:
            ep = ctx.enter_context
            constp = ep(tc.tile_pool(name="const", bufs=1))
            rgbp = ep(tc.tile_pool(name="rgb", bufs=1))
            tmpp = ep(tc.tile_pool(name="tmp", bufs=1))
            xfp = ep(tc.tile_pool(name="xf", bufs=2))
            gpadp = ep(tc.tile_pool(name="gpad", bufs=2))   # gpad + spdS share
            padp = ep(tc.tile_pool(name="pad", bufs=1))     # spad + npad share
            tplp = ep(tc.tile_pool(name="tpl", bufs=1))
            magp = ep(tc.tile_pool(name="mag", bufs=1))
            t16p = ep(tc.tile_pool(name="t16", bufs=1))     # gx/bigm, gy/keep, sprod/ht
            selp = ep(tc.tile_pool(name="sel", bufs=1))     # ax/fwd, ay/bwd
            mskp = ep(tc.tile_pool(name="msk", bufs=1))
            wkp = ep(tc.tile_pool(name="wk", bufs=2))
            curp = ep(tc.tile_pool(name="cur", bufs=3))
            xip = ep(tc.tile_pool(name="xi", bufs=4))
            ovp = ep(tc.tile_pool(name="ov", bufs=4))
            pvertp = ep(tc.tile_pool(name="pvert", bufs=2, space="PSUM"))
            pconvp = ep(tc.tile_pool(name="pconv", bufs=3, space="PSUM"))

            mats = constp.tile([128, N_MATS, 128], f16, tag="mats")
            nc.sync.dma_start(mats[:], mats_d.rearrange("m k n -> k m n"))
            convA = constp.tile([128, 4, 128], f16, tag="convA")
            nc.sync.dma_start(convA.rearrange("p g m -> p (g m)"), convA_d)
            convB = constp.tile([128, 4, 128], f16, tag="convB")
            nc.sync.dma_start(convB.rearrange("p g m -> p (g m)"), convB_d)
            brep = constp.tile([128, 4], f32, tag="brep")
            nc.sync.dma_start(brep[:], brep_d)
            zrow = constp.tile([128, 514], f16, tag="zrow")
            nc.vector.memset(zrow[:], 0.0)

            def mat(idx):
                return mats[:, idx, :]

            # eviction engine rotation for conv outputs (GPSIMD has no PSUM port)
            EV = ["act", "vec", "act", "vec", "act", "vec", "act", "act"]

            for bi in range(B_LOC):
                # ---- P1: load rgb, gray+floor -> gpad; fp16 hi/lo copies ----
                tr = rgbp.tile([128, NS, 512], f32, tag="tr")
                tg = rgbp.tile([128, NS, 512], f32, tag="tg")
                tb = rgbp.tile([128, NS, 512], f32, tag="tb")
                nc.sync.dma_start(tr[:], x_d[bi, 0].rearrange("(s p) j -> p s j", s=NS))
                nc.sync.dma_start(tg[:], x_d[bi, 1].rearrange("(s p) j -> p s j", s=NS))
                nc.sync.dma_start(tb[:], x_d[bi, 2].rearrange("(s p) j -> p s j", s=NS))
                g1 = tmpp.tile([128, NS, 512], f32, tag="ta")
                nc.vector.tensor_scalar(g1[:], tr[:], 0.2989, None, op0=op.mult)
                g2 = tmpp.tile([128, NS, 512], f32, tag="tb2")
                nc.scalar.activation(g2[:], tg[:], act.Copy, bias=0.0, scale=0.587)
                g3 = tmpp.tile([128, NS, 512], f32, tag="tc")
                nc.gpsimd.tensor_tensor(g3[:], g1[:], g2[:], op=op.add)
                g4 = tmpp.tile([128, NS, 512], f32, tag="ta")
                nc.scalar.activation(g4[:], tb[:], act.Copy, bias=0.0, scale=0.114)
                gray = tmpp.tile([128, NS, 512], f32, tag="td")
                nc.vector.tensor_tensor(gray[:], g3[:], g4[:], op=op.add)
                # floor via magic round + fixup; the two rounding steps sit on
                # different engines so bacc cannot fuse them
                y1 = tmpp.tile([128, NS, 512], f32, tag="tc")
                nc.vector.tensor_scalar(y1[:], gray[:], MAGIC_A, None, op0=op.add)
                z1 = tmpp.tile([128, NS, 512], f32, tag="tb2")
                nc.scalar.activation(z1[:], y1[:], act.Copy, bias=-MAGIC_B, scale=1.0)
                d1 = tmpp.tile([128, NS, 512], f32, tag="ta")
                nc.gpsimd.tensor_tensor(d1[:], gray[:], z1[:], op=op.subtract)
                gpad = gpadp.tile([128, NS, 514], f16, tag="gpad")
                nc.vector.scalar_tensor_tensor(
                    gpad[:, :, 1:513], d1[:], 1.0, z1[:], op0=op.is_ge, op1=op.add)
                nc.scalar.copy(gpad[:, :, 0:1], gpad[:, :, 1:2])
                nc.scalar.copy(gpad[:, :, 513:514], gpad[:, :, 512:513])

                # ---- P2: t = horizontal (1,2,1) smooth ----------------------
                u1 = t16p.tile([128, NS, 512], f16, tag="t16c")
                nc.vector.scalar_tensor_tensor(
                    u1[:], gpad[:, :, 1:513], 2.0, gpad[:, :, 0:512],
                    op0=op.mult, op1=op.add)
                tpl = tplp.tile([128, NS, 512], f16, tag="tpl")
                nc.gpsimd.tensor_tensor(tpl[:], u1[:], gpad[:, :, 2:514], op=op.add)

                # ---- P3: Sobel + mag + direction masks ----------------------
                spad = padp.tile([128, NS, 514], f16, tag="pad1")
                for s in range(NS):
                    ps = pvertp.tile([128, 512], f32, tag="pv")
                    tm = (I_T_TOP, I_T_MID, I_T_MID, I_T_BOT)[s]
                    nc.tensor.matmul(ps[:], mat(tm), gpad[:, s, 1:513],
                                     start=True, stop=False)
                    if s > 0:
                        nc.tensor.matmul(ps[:], mat(I_H_TOP), gpad[:, s - 1, 1:513],
                                         start=False, stop=(s == 3))
                    if s < 3:
                        nc.tensor.matmul(ps[:], mat(I_H_BOT), gpad[:, s + 1, 1:513],
                                         start=False, stop=True)
                    nc.scalar.copy(spad[:, s, 1:513], ps[:])
                nc.scalar.copy(spad[:, :, 0:1], spad[:, :, 1:2])
                nc.scalar.copy(spad[:, :, 513:514], spad[:, :, 512:513])
                gx = t16p.tile([128, NS, 512], f16, tag="t16a")
                nc.gpsimd.tensor_tensor(gx[:], spad[:, :, 2:514], spad[:, :, 0:512],
                                        op=op.subtract)
                gy = t16p.tile([128, NS, 512], f16, tag="t16b")
                for s in range(NS):
                    pg = pvertp.tile([128, 512], f32, tag="pv")
                    dm = (I_D_TOP, I_D_MID, I_D_MID, I_D_BOT)[s]
                    nc.tensor.matmul(pg[:], mat(dm), tpl[:, s, :],
                                     start=True, stop=False)
                    if s > 0:
                        nc.tensor.matmul(pg[:], mat(I_H_TOP_D), tpl[:, s - 1, :],
                                         start=False, stop=(s == 3))
                    if s < 3:
                        nc.tensor.matmul(pg[:], mat(I_H_BOT), tpl[:, s + 1, :],
                                         start=False, stop=True)
                    nc.scalar.copy(gy[:, s, :], pg[:])
                ax = selp.tile([128, NS, 512], f16, tag="sel1")
                nc.vector.scalar_tensor_tensor(ax[:], gx[:], -1.0, gx[:],
                                               op0=op.mult, op1=op.max)
                ay = selp.tile([128, NS, 512], f16, tag="sel2")
                nc.vector.scalar_tensor_tensor(ay[:], gy[:], -1.0, gy[:],
                                               op0=op.mult, op1=op.max)
                mag = magp.tile([128, NS, 514], f16, tag="mag")
                nc.gpsimd.tensor_tensor(mag[:, :, 1:513], ax[:], ay[:], op=op.add)
                nc.vector.memset(mag[:, :, 0:514:513], 0.0)
                hz = mskp.tile([128, NS, 512], u8, tag="hz")
                nc.vector.scalar_tensor_tensor(hz[:], ax[:], TG22, ay[:],
                                               op0=op.mult, op1=op.is_ge)
                vt = mskp.tile([128, NS, 512], u8, tag="vt")
                nc.vector.scalar_tensor_tensor(vt[:], ax[:], TG67, ay[:],
                                               op0=op.mult, op1=op.is_lt)
                sprod = t16p.tile([128, NS, 512], f16, tag="t16c")
                nc.gpsimd.tensor_tensor(sprod[:], gx[:], gy[:], op=op.mult)
                sn = mskp.tile([128, NS, 512], u8, tag="sn")
                nc.vector.tensor_scalar(sn[:], sprod[:], 0.0, None, op0=op.is_ge)

                # ---- P4: NMS via DMA-shifted neighbors ----------------------
                npad = padp.tile([128, NS, 514], f16, tag="pad1")
                nc.sync.dma_start(npad[1:128, :, :], mag[0:127, :, :])
                nc.sync.dma_start(npad[0:1, 1:NS, :], mag[127:128, 0:NS - 1, :])
                nc.sync.dma_start(npad[0:1, 0:1, :], zrow[0:1, :])
                spdS = gpadp.tile([128, NS, 514], f16, tag="gpad")
                nc.sync.dma_start(spdS[0:127, :, :], mag[1:128, :, :])
                nc.sync.dma_start(spdS[127:128, 0:NS - 1, :], mag[0:1, 1:NS, :])
                nc.sync.dma_start(spdS[127:128, NS - 1:NS, :], zrow[0:1, :])
                # fwd = where(horiz, e, where(vert, n, where(ssn, nw, ne)))
                fwd = selp.tile([128, NS, 512], f16, tag="sel1")
                nc.scalar.copy(fwd[:], npad[:, :, 2:514])                      # ne
                nc.vector.copy_predicated(fwd[:], sn[:], npad[:, :, 0:512])    # nw
                nc.vector.copy_predicated(fwd[:], vt[:], npad[:, :, 1:513])    # n
                nc.vector.copy_predicated(fwd[:], hz[:], mag[:, :, 2:514])     # e
                bwd = selp.tile([128, NS, 512], f16, tag="sel2")
                nc.scalar.copy(bwd[:], spdS[:, :, 0:512])                      # sw
                nc.vector.copy_predicated(bwd[:], sn[:], spdS[:, :, 2:514])    # se
                nc.vector.copy_predicated(bwd[:], vt[:], spdS[:, :, 1:513])    # s
                nc.vector.copy_predicated(bwd[:], hz[:], mag[:, :, 0:512])     # w
                bigm = t16p.tile([128, NS, 512], f16, tag="t16a")
                nc.vector.scalar_tensor_tensor(bigm[:], fwd[:], 1.0, bwd[:],
                                               op0=op.add, op1=op.max)
                keep = t16p.tile([128, NS, 512], f16, tag="t16b")
                nc.vector.tensor_tensor(keep[:], mag[:, :, 1:513], bigm[:],
                                        op=op.is_ge)
                cur = curp.tile([128, NS, 514], f16, tag="cpad")
                nc.vector.scalar_tensor_tensor(
                    cur[:, :, 1:513], mag[:, :, 1:513], 150.0, keep[:],
                    op0=op.is_gt, op1=op.mult)
                nc.vector.memset(cur[:, :, 0:514:513], 0.0)
                wk = wkp.tile([128, NS, 512], f16, tag="wk")
                nc.vector.scalar_tensor_tensor(
                    wk[:], mag[:, :, 1:513], 50.0, keep[:], op0=op.is_gt, op1=op.mult)

                # ---- P5: hysteresis, 3 iterations ---------------------------
                for it in range(3):
                    h1 = selp.tile([128, NS, 512], f16, tag="sel1")
                    nc.gpsimd.tensor_tensor(h1[:], cur[:, :, 0:512],
                                            cur[:, :, 2:514], op=op.add)
                    ht = t16p.tile([128, NS, 512], f16, tag="t16c")
                    nc.vector.tensor_tensor(ht[:], h1[:], cur[:, :, 1:513], op=op.add)
                    cnew = curp.tile([128, NS, 514], f16, tag="cpad")
                    for s in range(NS):
                        pv = pvertp.tile([128, 512], f32, tag="pv")
                        nc.tensor.matmul(pv[:], mat(I_V), ht[:, s, :],
                                         start=True, stop=False)
                        if s > 0:
                            nc.tensor.matmul(pv[:], mat(I_H_TOP), ht[:, s - 1, :],
                                             start=False, stop=(s == 3))
                        if s < 3:
                            nc.tensor.matmul(pv[:], mat(I_H_BOT), ht[:, s + 1, :],
                                             start=False, stop=True)
                        nc.vector.scalar_tensor_tensor(
                            cnew[:, s, 1:513], pv[:], 0.0, wk[:, s, :],
                            op0=op.is_gt, op1=op.mult)
                    nc.vector.memset(cnew[:, :, 0:514:513], 0.0)
                    cur = cnew

                # edge channel round-trips through DRAM to reach the folded
                # [rr, w, j] conv layout (partition<->free exchange needs
                # a DRAM AP)
                for s in range(NS):
                    nc.sync.dma_start(e_d[bi, 128 * s:128 * s + 128, :],
                                      cur[:, s, 1:513])

                # ---- P6: conv + output --------------------------------------
                for s in range(NS):
                    xf = xfp.tile([64, 8, 512], f32, tag="xf")
                    for c in range(3):
                        nc.sync.dma_start(
                            xf[16 * c:16 * c + 16],
                            x_d[bi, c, 128 * s:128 * s + 128, :]
                                .rearrange("(w r) j -> r w j", w=8))
                    # partitions 48-63: filler (overwritten by the edge rows
                    # in xi) so the 64-partition convert ops see real floats
                    nc.sync.dma_start(
                        xf[48:64],
                        x_d[bi, 0, 128 * s:128 * s + 128, :]
                            .rearrange("(w r) j -> r w j", w=8))
                    xi = xip.tile([128, 8, 512], f16, tag="xi")
                    nc.scalar.activation(xi[0:64], xf[:], act.Copy,
                                         bias=0.0, scale=1.0)
                    nc.vector.tensor_tensor(xi[64:128], xf[:], xi[0:64],
                                            op=op.subtract)
                    nc.sync.dma_start(
                        xi[48:64],
                        e_d[bi, 128 * s:128 * s + 128, :]
                            .rearrange("(w r) j -> r w j", w=8))
                    nc.sync.dma_start(
                        xi[112:128],
                        e_d[bi, 128 * s:128 * s + 128, :]
                            .rearrange("(w r) j -> r w j", w=8))
                    for og in range(4):
                        for vg in range(2):  # window groups (v0,v1), (v2,v3)
                            pcs = []
                            for v in (2 * vg, 2 * vg + 1):
                                pc = pconvp.tile([128, 1024], f32, tag="pc")
                                nc.tensor.matmul(pc[:, 0:512], convA[:, og, :],
                                                 xi[:, 2 * v, :],
                                                 start=True, stop=False)
                                nc.tensor.matmul(pc[:, 512:1024], convA[:, og, :],
                                                 xi[:, 2 * v + 1, :],
                                                 start=True, stop=False)
                                pcs.append(pc)
                            for i, v in enumerate((2 * vg, 2 * vg + 1)):
                                pc = pcs[i]
                                nc.tensor.matmul(pc[:, 0:512], convB[:, og, :],
                                                 xi[:, 2 * v, :],
                                                 start=False, stop=True)
                                nc.tensor.matmul(pc[:, 512:1024], convB[:, og, :],
                                                 xi[:, 2 * v + 1, :],
                                                 start=False, stop=True)
                            for i, v in enumerate((2 * vg, 2 * vg + 1)):
                                pc = pcs[i]
                                ov = ovp.tile([128, 1024], f16, tag="ov")
                                eng = EV[(2 * og + (s + v)) % 8]
                                if eng == "act":
                                    nc.scalar.activation(
                                        ov[:], pc[:], act.Relu,
                                        bias=brep[:, og:og + 1], scale=1.0)
                                elif eng == "vec":
                                    nc.vector.tensor_scalar(
                                        ov[:], pc[:], brep[:, og:og + 1], 0.0,
                                        op0=op.add, op1=op.max)
                                else:
                                    nc.gpsimd.tensor_scalar(
                                        ov[:], pc[:], brep[:, og:og + 1], 0.0,
                                        op0=op.add, op1=op.max)
                                r0 = 128 * s + 32 * v
                                nc.sync.dma_start(
                                    out_d[bi][8 * og:8 * og + 8, r0:r0 + 16, :],
                                    ov[:, 0:512])
                                nc.sync.dma_start(
                                    out_d[bi][8 * og:8 * og + 8, r0 + 16:r0 + 32, :],
                                    ov[:, 512:1024])
    nc.compile()
    return nc


def _get_program():
    if "nc" not in _PROG_CACHE:
        _PROG_CACHE["nc"] = build_program()
    return _PROG_CACHE["nc"]


def build_in_maps(x, W, b):
    x = np.ascontiguousarray(np.asarray(x, dtype=np.float32))
    W = np.asarray(W, dtype=np.float32)
    b = np.asarray(b, dtype=np.float32)
    mats = build_shift_mats()
    cA, cB, brep = build_conv_banks(W, b)
    cA = np.ascontiguousarray(cA.reshape(128, 512))
    cB = np.ascontiguousarray(cB.reshape(128, 512))
    in_maps = []
    for core in range(N_CORES):
        xs = np.ascontiguousarray(x[B_LOC * core:B_LOC * (core + 1)])
        in_maps.append({"x": xs, "mats": mats, "convA": cA, "convB": cB,
                        "brep": brep})
    return in_maps


def kernel(x: np.ndarray, W: np.ndarray, b: np.ndarray) -> np.ndarray:
    from concourse.bass_utils import run_bass_kernel_spmd

    nc = _get_program()
    in_maps = build_in_maps(x, W, b)
    res = run_bass_kernel_spmd(nc, in_maps, core_ids=list(range(N_CORES)))
    return np.concatenate(
        [r["out"].astype(np.float32) for r in res.results], axis=0)


# revision 18
# speedup vs baseline: 1.5661x; 1.5661x over previous
"""Trainium2 Bass kernel for grayscale+Canny+1x1-conv (nn_BFA_3015067042007).

Data-parallel over batch: 16 images -> 8 cores x 2 images.

Design (v4):
  - Whole-image tiles [128, 4, 514]: the 4 row-strips of an image are packed
    along the free dim, so every elementwise Canny op covers the full image
    in ONE instruction (amortizes per-instruction overhead 4x). Stencil
    matmuls run per strip on free-dim slices of the big tiles; strip-halo
    rows enter via 1-row shift matrices accumulated in PSUM.
  - Fused 2-pass fp16 conv: rhs tile per strip is [128, 8, 512] with
    pseudo-channels [Rh Gh Bh E | Rl Gl Bl E'] (hi/lo fp16 split of x along
    K). Pass A lhsT carries Wh on all rgb rows and (Wh_e, Wl_e) on the two
    edge-row groups; pass B carries Wl on rgb rows. Sum = (Wh+Wl)(xh+xl)
    + W_e*e with no dropped cross terms -> fp32-level accuracy with 64
    matmuls/strip (vs 96 for the 3-pass hi/lo scheme).
  - The hi/lo split of x is prepared host-side (operand packing, like the
    lhsT banks) in the exact [128, 8*512] folded layout, so each strip's
    conv rhs is ONE fully-contiguous 1MB DMA (8KB descriptors).
  - Edge channel rows round-trip through a DRAM scratch to reach the folded
    layout (partition<->free exchange requires a DRAM AP).
  - fp16 output (rel err ~5e-4 << 2e-2 tolerance) in an eviction-native
    DRAM layout [B, s, og, v, 128, 1024] -> every output store is one fully
    contiguous 256KB DMA (2KB descriptors); the host untangles + upcasts.
    Halves HBM write traffic AND keeps HWDGE descriptor generation (the v3
    bottleneck: ~6ns/descriptor, serialized on one queue) off the critical
    path. ACT-eviction stores issue on the scalar HWDGE ring (hidden under
    ACT ALU time), DVE-eviction stores on the sync ring.
"""

import numpy as np

B_FULL = 16
N_CORES = 8
B_LOC = B_FULL // N_CORES
H = 512
W_IMG = 512
NS = 4  # row strips of 128

MAGIC_A = 8388607.5
MAGIC_B = 8388608.0
TG22 = 0.4142135623730951
TG67 = 2.414213562373095

# shift-matrix stack indices
I_T_TOP, I_T_MID, I_T_BOT = 0, 1, 2
I_D_TOP, I_D_MID, I_D_BOT = 3, 4, 5
I_V, I_H_TOP, I_H_BOT, I_H_TOP_D = 6, 7, 8, 9
I_N, I_S = 10, 11
N_MATS = 12


def build_shift_mats():
    m = np.zeros((N_MATS, 128, 128), np.float16)
    i = np.arange(128)
    # vertical (1,2,1) smooth: out[p] = in[p-1] + 2 in[p] + in[p+1]
    for t in (I_T_TOP, I_T_MID, I_T_BOT):
        m[t][i, i] = 2.0
        m[t][i[:-1], i[1:]] = 1.0
        m[t][i[1:], i[:-1]] = 1.0
    m[I_T_TOP][0, 0] = 3.0      # replicate pad at image top
    m[I_T_BOT][127, 127] = 3.0  # replicate pad at image bottom
    # vertical diff: out[p] = in[p+1] - in[p-1]
    for t in (I_D_TOP, I_D_MID, I_D_BOT):
        m[t][i[1:], i[:-1]] = 1.0
        m[t][i[:-1], i[1:]] = -1.0
    m[I_D_TOP][0, 0] = -1.0       # out[0] = in[1] - in[0]
    m[I_D_BOT][127, 127] = 1.0    # out[127] = in[127] - in[126]
    m[I_V][i, i] = 1.0            # vertical (1,1,1) sum
    m[I_V][i[:-1], i[1:]] = 1.0
    m[I_V][i[1:], i[:-1]] = 1.0
    m[I_H_TOP][127, 0] = 1.0      # prev strip row 127 -> out row 0
    m[I_H_BOT][0, 127] = 1.0      # next strip row 0 -> out row 127
    m[I_H_TOP_D][127, 0] = -1.0   # diff halo: -in_prev[127]
    m[I_N][i[:-1], i[1:]] = 1.0   # out[p] = in[p-1]
    m[I_S][i[1:], i[:-1]] = 1.0   # out[p] = in[p+1]
    return m


def build_conv_banks(W, b):
    """Two fp16 lhsT banks [128, 4, 128] for the fused 2-pass conv.

    rhs tile partition p: hi rgb 16c+rr (c<3), hi edge 48+rr, lo rgb
    64+16c+rr, lo edge 112+rr; rr = row within the 16-row window.
    psum m = 16*oi + rr, och = 8*og + oi.
    Pass A: Wh on all rgb rows; edge rows carry (Wh_e, Wl_e).
    Pass B: Wl on all rgb rows; edge rows 0."""
    Wc = W.astype(np.float32).copy()
    Wc[:, 3] = Wc[:, 3] * np.float32(255.0)
    Wh = Wc.astype(np.float16)
    Wl = (Wc - Wh.astype(np.float32)).astype(np.float16)
    convA = np.zeros((128, 4, 128), np.float16)
    convB = np.zeros((128, 4, 128), np.float16)
    rr = np.arange(16)
    for og in range(4):
        for oi in range(8):
            oc = 8 * og + oi
            m = 16 * oi + rr
            for c in range(3):
                convA[16 * c + rr, og, m] = Wh[oc, c]
                convA[64 + 16 * c + rr, og, m] = Wh[oc, c]
                convB[16 * c + rr, og, m] = Wl[oc, c]
                convB[64 + 16 * c + rr, og, m] = Wl[oc, c]
            convA[48 + rr, og, m] = Wh[oc, 3]
            convA[112 + rr, og, m] = Wl[oc, 3]
    p = np.arange(128)
    brep = np.stack([b[8 * og + p // 16] for og in range(4)], axis=1)
    return convA, convB, brep.astype(np.float32)


def build_xfold(x):
    """Host-side conv operand packing: fp16 hi/lo split of x in the exact
    [s, 128, 8, 512] folded rhs layout (edge rows zero, filled on-chip)."""
    B = x.shape[0]
    xh = x.astype(np.float16)
    xl = (x - xh.astype(np.float32)).astype(np.float16)
    xfold = np.zeros((B, NS, 128, 8, 512), np.float16)
    # [B,3,512,512] -> [B,3,s,w,rr,j] -> [B,s,c,rr,w,j]
    for src, base in ((xh, 0), (xl, 64)):
        t = src.reshape(B, 3, NS, 8, 16, 512).transpose(0, 2, 1, 4, 3, 5)
        xfold[:, :, base:base + 48] = t.reshape(B, NS, 48, 8, 512)
    return np.ascontiguousarray(xfold)


_PROG_CACHE = {}


def build_program():
    import concourse.bacc as bacc
    import concourse.tile as tile
    import concourse.mybir as mybir
    from concourse.mybir import AluOpType as op, ActivationFunctionType as act
    from contextlib import ExitStack

    f32 = mybir.dt.float32
    f16 = mybir.dt.float16
    u8 = mybir.dt.uint8

    nc = bacc.Bacc("TRN2", target_bir_lowering=False, debug=False)
    x_d = nc.dram_tensor("x", [B_LOC, 3, H, W_IMG], f32, kind="ExternalInput").ap()
    xfold_d = nc.dram_tensor("xfold", [B_LOC, NS, 128, 8, 512], f16,
                             kind="ExternalInput").ap()
    mats_d = nc.dram_tensor("mats", [N_MATS, 128, 128], f16, kind="ExternalInput").ap()
    convA_d = nc.dram_tensor("convA", [128, 512], f16, kind="ExternalInput").ap()
    convB_d = nc.dram_tensor("convB", [128, 512], f16, kind="ExternalInput").ap()
    brep_d = nc.dram_tensor("brep", [128, 4], f32, kind="ExternalInput").ap()
    out_d = nc.dram_tensor("out", [B_LOC, NS, 4, 4, 128, 1024], f16,
                           kind="ExternalOutput").ap()
    e_d = nc.dram_tensor("escratch", [B_LOC, H, W_IMG], f16, kind="Internal").ap()

    with tile.TileContext(nc) as tc:
        with ExitStack() as ctx:
            ep = ctx.enter_context
            constp = ep(tc.tile_pool(name="const", bufs=1))
            rgbp = ep(tc.tile_pool(name="rgb", bufs=1))
            tmpp = ep(tc.tile_pool(name="tmp", bufs=1))
            gpadp = ep(tc.tile_pool(name="gpad", bufs=2))   # gpad + spdS share
            padp = ep(tc.tile_pool(name="pad", bufs=1))     # spad + npad share
            tplp = ep(tc.tile_pool(name="tpl", bufs=1))
            magp = ep(tc.tile_pool(name="mag", bufs=1))
            t16p = ep(tc.tile_pool(name="t16", bufs=1))     # gx/bigm, gy/keep, u1/sprod/ht
            selp = ep(tc.tile_pool(name="sel", bufs=1))     # ax/fwd, ay/bwd
            mskp = ep(tc.tile_pool(name="msk", bufs=1))
            wkp = ep(tc.tile_pool(name="wk", bufs=2))
            curp = ep(tc.tile_pool(name="cur", bufs=3))
            xip = ep(tc.tile_pool(name="xi", bufs=3))
            ovp = ep(tc.tile_pool(name="ov", bufs=6))
            pvertp = ep(tc.tile_pool(name="pvert", bufs=2, space="PSUM"))
            pconvp = ep(tc.tile_pool(name="pconv", bufs=3, space="PSUM"))

            mats = constp.tile([128, N_MATS, 128], f16, tag="mats")
            nc.sync.dma_start(mats[:], mats_d.rearrange("m k n -> k m n"))
            convA = constp.tile([128, 4, 128], f16, tag="convA")
            nc.sync.dma_start(convA.rearrange("p g m -> p (g m)"), convA_d)
            convB = constp.tile([128, 4, 128], f16, tag="convB")
            nc.sync.dma_start(convB.rearrange("p g m -> p (g m)"), convB_d)
            brep = constp.tile([128, 4], f32, tag="brep")
            nc.sync.dma_start(brep[:], brep_d)

            def mat(idx):
                return mats[:, idx, :]

            # eviction engine rotation (GPSIMD has no PSUM port)
            EV = ["act", "vec", "act", "act", "vec", "act", "vec", "act"]

            for bi in range(B_LOC):
                # ---- P1: load rgb, gray+floor -> gpad -----------------------
                tr = rgbp.tile([128, NS, 512], f32, tag="tr")
                tg = rgbp.tile([128, NS, 512], f32, tag="tg")
                tb = rgbp.tile([128, NS, 512], f32, tag="tb")
                nc.sync.dma_start(tr[:], x_d[bi, 0].rearrange("(s p) j -> p s j", s=NS))
                nc.sync.dma_start(tg[:], x_d[bi, 1].rearrange("(s p) j -> p s j", s=NS))
                nc.sync.dma_start(tb[:], x_d[bi, 2].rearrange("(s p) j -> p s j", s=NS))
                g1 = tmpp.tile([128, NS, 512], f32, tag="ta")
                nc.vector.tensor_scalar(g1[:], tr[:], 0.2989, None, op0=op.mult)
                g2 = tmpp.tile([128, NS, 512], f32, tag="tb2")
                nc.scalar.activation(g2[:], tg[:], act.Copy, bias=0.0, scale=0.587)
                g3 = tmpp.tile([128, NS, 512], f32, tag="tc")
                nc.gpsimd.tensor_tensor(g3[:], g1[:], g2[:], op=op.add)
                g4 = tmpp.tile([128, NS, 512], f32, tag="ta")
                nc.scalar.activation(g4[:], tb[:], act.Copy, bias=0.0, scale=0.114)
                gray = tmpp.tile([128, NS, 512], f32, tag="td")
                nc.vector.tensor_tensor(gray[:], g3[:], g4[:], op=op.add)
                # floor via magic round + fixup; the two rounding steps sit on
                # different engines so bacc cannot fuse them
                y1 = tmpp.tile([128, NS, 512], f32, tag="tc")
                nc.vector.tensor_scalar(y1[:], gray[:], MAGIC_A, None, op0=op.add)
                z1 = tmpp.tile([128, NS, 512], f32, tag="tb2")
                nc.scalar.activation(z1[:], y1[:], act.Copy, bias=-MAGIC_B, scale=1.0)
                d1 = tmpp.tile([128, NS, 512], f32, tag="ta")
                nc.gpsimd.tensor_tensor(d1[:], gray[:], z1[:], op=op.subtract)
                gpad = gpadp.tile([128, NS, 514], f16, tag="gpad")
                nc.vector.scalar_tensor_tensor(
                    gpad[:, :, 1:513], d1[:], 1.0, z1[:], op0=op.is_ge, op1=op.add)
                nc.scalar.copy(gpad[:, :, 0:1], gpad[:, :, 1:2])
                nc.scalar.copy(gpad[:, :, 513:514], gpad[:, :, 512:513])

                # ---- P2: t = horizontal (1,2,1) smooth ----------------------
                u1 = t16p.tile([128, NS, 512], f16, tag="t16c")
                nc.vector.scalar_tensor_tensor(
                    u1[:], gpad[:, :, 1:513], 2.0, gpad[:, :, 0:512],
                    op0=op.mult, op1=op.add)
                tpl = tplp.tile([128, NS, 512], f16, tag="tpl")
                nc.gpsimd.tensor_tensor(tpl[:], u1[:], gpad[:, :, 2:514], op=op.add)

                # ---- P3: Sobel + mag + direction masks ----------------------
                spad = padp.tile([128, NS, 514], f16, tag="pad1")
                for s in range(NS):
                    ps = pvertp.tile([128, 512], f32, tag="pv")
                    tm = (I_T_TOP, I_T_MID, I_T_MID, I_T_BOT)[s]
                    nc.tensor.matmul(ps[:], mat(tm), gpad[:, s, 1:513],
                                     start=True, stop=False)
                    if s > 0:
                        nc.tensor.matmul(ps[:], mat(I_H_TOP), gpad[:, s - 1, 1:513],
                                         start=False, stop=(s == 3))
                    if s < 3:
                        nc.tensor.matmul(ps[:], mat(I_H_BOT), gpad[:, s + 1, 1:513],
                                         start=False, stop=True)
                    nc.scalar.copy(spad[:, s, 1:513], ps[:])
                nc.scalar.copy(spad[:, :, 0:1], spad[:, :, 1:2])
                nc.scalar.copy(spad[:, :, 513:514], spad[:, :, 512:513])
                gx = t16p.tile([128, NS, 512], f16, tag="t16a")
                nc.gpsimd.tensor_tensor(gx[:], spad[:, :, 2:514], spad[:, :, 0:512],
                                        op=op.subtract)
                gy = t16p.tile([128, NS, 512], f16, tag="t16b")
                for s in range(NS):
                    pg = pvertp.tile([128, 512], f32, tag="pv")
                    dm = (I_D_TOP, I_D_MID, I_D_MID, I_D_BOT)[s]
                    nc.tensor.matmul(pg[:], mat(dm), tpl[:, s, :],
                                     start=True, stop=False)
                    if s > 0:
                        nc.tensor.matmul(pg[:], mat(I_H_TOP_D), tpl[:, s - 1, :],
                                         start=False, stop=(s == 3))
                    if s < 3:
                        nc.tensor.matmul(pg[:], mat(I_H_BOT), tpl[:, s + 1, :],
                                         start=False, stop=True)
                    if s % 2 == 0:
                        nc.scalar.copy(gy[:, s, :], pg[:])
                    else:
                        nc.vector.tensor_copy(gy[:, s, :], pg[:])
                ax = selp.tile([128, NS, 512], f16, tag="sel1")
                nc.scalar.activation(ax[:], gx[:], act.Abs)
                ay = selp.tile([128, NS, 512], f16, tag="sel2")
                nc.scalar.activation(ay[:], gy[:], act.Abs)
                mag = magp.tile([128, NS, 514], f16, tag="mag")
                nc.gpsimd.tensor_tensor(mag[:, :, 1:513], ax[:], ay[:], op=op.add)
                nc.vector.memset(mag[:, :, 0:514:513], 0.0)
                hz = mskp.tile([128, NS, 512], u8, tag="hz")
                nc.vector.scalar_tensor_tensor(hz[:], ax[:], TG22, ay[:],
                                               op0=op.mult, op1=op.is_ge)
                vt = mskp.tile([128, NS, 512], u8, tag="vt")
                nc.vector.scalar_tensor_tensor(vt[:], ax[:], TG67, ay[:],
                                               op0=op.mult, op1=op.is_lt)
                sprod = t16p.tile([128, NS, 512], f16, tag="t16c")
                nc.gpsimd.tensor_tensor(sprod[:], gx[:], gy[:], op=op.mult)
                sn = mskp.tile([128, NS, 512], u8, tag="sn")
                nc.vector.tensor_scalar(sn[:], sprod[:], 0.0, None, op0=op.is_ge)

                # ---- P4: N/S neighbor shifts (PE) + NMS ---------------------
                npad = padp.tile([128, NS, 514], f16, tag="pad1")
                for s in range(NS):
                    pn = pvertp.tile([128, 512], f32, tag="pv")
                    nc.tensor.matmul(pn[:], mat(I_N), mag[:, s, 1:513],
                                     start=True, stop=(s == 0))
                    if s > 0:
                        nc.tensor.matmul(pn[:], mat(I_H_TOP), mag[:, s - 1, 1:513],
                                         start=False, stop=True)
                    if s % 2 == 0:
                        nc.scalar.copy(npad[:, s, 1:513], pn[:])
                    else:
                        nc.vector.tensor_copy(npad[:, s, 1:513], pn[:])
                nc.vector.memset(npad[:, :, 0:514:513], 0.0)
                spdS = gpadp.tile([128, NS, 514], f16, tag="gpad")
                for s in range(NS):
                    pss = pvertp.tile([128, 512], f32, tag="pv")
                    nc.tensor.matmul(pss[:], mat(I_S), mag[:, s, 1:513],
                                     start=True, stop=(s == 3))
                    if s < 3:
                        nc.tensor.matmul(pss[:], mat(I_H_BOT), mag[:, s + 1, 1:513],
                                         start=False, stop=True)
                    if s % 2 == 0:
                        nc.vector.tensor_copy(spdS[:, s, 1:513], pss[:])
                    else:
                        nc.scalar.copy(spdS[:, s, 1:513], pss[:])
                nc.vector.memset(spdS[:, :, 0:514:513], 0.0)
                # fwd = where(horiz, e, where(vert, n, where(ssn, nw, ne)))
                fwd = selp.tile([128, NS, 512], f16, tag="sel1")
                nc.scalar.copy(fwd[:], npad[:, :, 2:514])                      # ne
                nc.vector.copy_predicated(fwd[:], sn[:], npad[:, :, 0:512])    # nw
                nc.vector.copy_predicated(fwd[:], vt[:], npad[:, :, 1:513])    # n
                nc.vector.copy_predicated(fwd[:], hz[:], mag[:, :, 2:514])     # e
                bwd = selp.tile([128, NS, 512], f16, tag="sel2")
                nc.scalar.copy(bwd[:], spdS[:, :, 0:512])                      # sw
                nc.vector.copy_predicated(bwd[:], sn[:], spdS[:, :, 2:514])    # se
                nc.vector.copy_predicated(bwd[:], vt[:], spdS[:, :, 1:513])    # s
                nc.vector.copy_predicated(bwd[:], hz[:], mag[:, :, 0:512])     # w
                bigm = t16p.tile([128, NS, 512], f16, tag="t16a")
                nc.vector.scalar_tensor_tensor(bigm[:], fwd[:], 1.0, bwd[:],
                                               op0=op.add, op1=op.max)
                keep = t16p.tile([128, NS, 512], f16, tag="t16b")
                nc.vector.tensor_tensor(keep[:], mag[:, :, 1:513], bigm[:],
                                        op=op.is_ge)
                cur = curp.tile([128, NS, 514], f16, tag="cpad")
                nc.vector.scalar_tensor_tensor(
                    cur[:, :, 1:513], mag[:, :, 1:513], 150.0, keep[:],
                    op0=op.is_gt, op1=op.mult)
                nc.vector.memset(cur[:, :, 0:514:513], 0.0)
                wk = wkp.tile([128, NS, 512], f16, tag="wk")
                nc.vector.scalar_tensor_tensor(
                    wk[:], mag[:, :, 1:513], 50.0, keep[:], op0=op.is_gt, op1=op.mult)

                # ---- P5: hysteresis, 3 iterations ---------------------------
                for it in range(3):
                    h1 = selp.tile([128, NS, 512], f16, tag="sel1")
                    nc.gpsimd.tensor_tensor(h1[:], cur[:, :, 0:512],
                                            cur[:, :, 2:514], op=op.add)
                    ht = t16p.tile([128, NS, 512], f16, tag="t16c")
                    nc.vector.tensor_tensor(ht[:], h1[:], cur[:, :, 1:513], op=op.add)
                    cnew = curp.tile([128, NS, 514], f16, tag="cpad")
                    for s in range(NS):
                        pv = pvertp.tile([128, 512], f32, tag="pv")
                        nc.tensor.matmul(pv[:], mat(I_V), ht[:, s, :],
                                         start=True, stop=False)
                        if s > 0:
                            nc.tensor.matmul(pv[:], mat(I_H_TOP), ht[:, s - 1, :],
                                             start=False, stop=(s == 3))
                        if s < 3:
                            nc.tensor.matmul(pv[:], mat(I_H_BOT), ht[:, s + 1, :],
                                             start=False, stop=True)
                        nc.vector.scalar_tensor_tensor(
                            cnew[:, s, 1:513], pv[:], 0.0, wk[:, s, :],
                            op0=op.is_gt, op1=op.mult)
                    nc.vector.memset(cnew[:, :, 0:514:513], 0.0)
                    cur = cnew

                # edge channel round-trips through DRAM to reach the folded
                # conv layout (partition<->free exchange needs a DRAM AP)
                nc.sync.dma_start(e_d[bi].rearrange("(s p) j -> p s j", s=NS),
                                  cur[:, :, 1:513])

                # ---- P6: conv + output --------------------------------------
                for s in range(NS):
                    xi = xip.tile([128, 8, 512], f16, tag="xi")
                    nc.sync.dma_start(xi[:], xfold_d[bi, s])
                    nc.sync.dma_start(
                        xi[48:64],
                        e_d[bi, 128 * s:128 * s + 128, :]
                            .rearrange("(w r) j -> r w j", w=8))
                    nc.sync.dma_start(
                        xi[112:128],
                        e_d[bi, 128 * s:128 * s + 128, :]
                            .rearrange("(w r) j -> r w j", w=8))
                    for og in range(4):
                        for vg in range(2):  # window groups (v0,v1), (v2,v3)
                            pcs = []
                            for v in (2 * vg, 2 * vg + 1):
                                pc = pconvp.tile([128, 1024], f32, tag="pc")
                                nc.tensor.matmul(pc[:, 0:512], convA[:, og, :],
                                                 xi[:, 2 * v, :],
                                                 start=True, stop=False)
                                nc.tensor.matmul(pc[:, 512:1024], convA[:, og, :],
                                                 xi[:, 2 * v + 1, :],
                                                 start=True, stop=False)
                                pcs.append(pc)
                            for i, v in enumerate((2 * vg, 2 * vg + 1)):
                                pc = pcs[i]
                                nc.tensor.matmul(pc[:, 0:512], convB[:, og, :],
                                                 xi[:, 2 * v, :],
                                                 start=False, stop=True)
                                nc.tensor.matmul(pc[:, 512:1024], convB[:, og, :],
                                                 xi[:, 2 * v + 1, :],
                                                 start=False, stop=True)
                            for i, v in enumerate((2 * vg, 2 * vg + 1)):
                                pc = pcs[i]
                                ov = ovp.tile([128, 1024], f16, tag="ov")
                                eng = EV[(2 * og + s + v) % 8]
                                if eng == "act":
                                    nc.scalar.activation(
                                        ov[:], pc[:], act.Relu,
                                        bias=brep[:, og:og + 1], scale=1.0)
                                    nc.scalar.dma_start(out_d[bi, s, og, v], ov[:])
                                else:
                                    nc.vector.tensor_scalar(
                                        ov[:], pc[:], brep[:, og:og + 1], 0.0,
                                        op0=op.add, op1=op.max)
                                    nc.sync.dma_start(out_d[bi, s, og, v], ov[:])
    nc.compile()
    return nc


def _get_program():
    if "nc" not in _PROG_CACHE:
        _PROG_CACHE["nc"] = build_program()
    return _PROG_CACHE["nc"]


def build_in_maps(x, W, b):
    x = np.ascontiguousarray(np.asarray(x, dtype=np.float32))
    W = np.asarray(W, dtype=np.float32)
    b = np.asarray(b, dtype=np.float32)
    mats = build_shift_mats()
    cA, cB, brep = build_conv_banks(W, b)
    cA = np.ascontiguousarray(cA.reshape(128, 512))
    cB = np.ascontiguousarray(cB.reshape(128, 512))
    xfold = build_xfold(x)
    in_maps = []
    for core in range(N_CORES):
        lo, hi = B_LOC * core, B_LOC * (core + 1)
        in_maps.append({"x": np.ascontiguousarray(x[lo:hi]),
                        "xfold": np.ascontiguousarray(xfold[lo:hi]),
                        "mats": mats, "convA": cA, "convB": cB, "brep": brep})
    return in_maps


def untangle(raw):
    """[B_LOC, s, og, v, 128, 1024] f16 -> [B_LOC, 32, 512, 512] f32."""
    r = raw.reshape(B_LOC, NS, 4, 4, 8, 16, 2, 512)
    r = r.transpose(0, 2, 4, 1, 3, 6, 5, 7)  # bi, og, oi, s, v, v2, rr, j
    return r.reshape(B_LOC, 32, H, W_IMG).astype(np.float32)


def kernel(x: np.ndarray, W: np.ndarray, b: np.ndarray) -> np.ndarray:
    from concourse.bass_utils import run_bass_kernel_spmd

    nc = _get_program()
    in_maps = build_in_maps(x, W, b)
    res = run_bass_kernel_spmd(nc, in_maps, core_ids=list(range(N_CORES)))
    return np.concatenate([untangle(r["out"]) for r in res.results], axis=0)


# revision 20
# speedup vs baseline: 1.6384x; 1.0461x over previous
"""Trainium2 Bass kernel for grayscale+Canny+1x1-conv (nn_BFA_3015067042007).

Data-parallel over batch: 16 images -> 8 cores x 2 images.

Design (v5):
  - Whole-image tiles [128, 4, 514]: the 4 row-strips of an image are packed
    along the free dim, so every elementwise Canny op covers the full image
    in ONE instruction (amortizes per-instruction overhead 4x). Stencil
    matmuls run per strip on free-dim slices of the big tiles; strip-halo
    rows enter via 1-row shift matrices accumulated in PSUM.
  - Software-pipelined emission: the two images' phases are emitted
    staggered (P1-P3 of img1 between the NMS/hysteresis and conv of img0)
    so every engine queue always holds ready work from the other image.
  - Fused 2-pass fp16 conv: rhs tile per strip is [128, 8, 512] with
    pseudo-channels [Rh Gh Bh E | Rl Gl Bl E'] (hi/lo fp16 split of x along
    K). Pass A lhsT carries Wh on all rgb rows and (Wh_e, Wl_e) on the two
    edge-row groups; pass B carries Wl on rgb rows. Sum = (Wh+Wl)(xh+xl)
    + W_e*e with no dropped cross terms -> fp32-level accuracy with 64
    matmuls/strip (vs 96 for the 3-pass hi/lo scheme).
  - The hi/lo split of x is prepared host-side (operand packing, like the
    lhsT banks) in the exact [128, 8*512] folded layout, so each strip's
    conv rhs is ONE fully-contiguous 1MB DMA (8KB descriptors).
  - Edge channel rows round-trip through a DRAM scratch to reach the folded
    layout (partition<->free exchange requires a DRAM AP).
  - fp16 output (rel err ~5e-4 << 2e-2 tolerance) in an eviction-native
    DRAM layout [B, s, og, v, 128, 1024] -> every output store is one fully
    contiguous 256KB DMA (2KB descriptors); the host untangles + upcasts.
    Halves HBM write traffic AND keeps HWDGE descriptor generation (the v3
    bottleneck: ~6ns/descriptor, serialized on one queue) off the critical
    path. ACT-eviction stores issue on the scalar HWDGE ring (hidden under
    ACT ALU time), DVE-eviction stores on the sync ring.
  - strong/weak thresholds fused with NMS via integer identity:
    cur = (mag >= max(bigm, 151)), wk = (mag >= max(bigm, 51)) -- one
    scalar_tensor_tensor each instead of an is_ge tensor_tensor + two stt.
"""

import numpy as np

B_FULL = 16
N_CORES = 8
B_LOC = B_FULL // N_CORES
H = 512
W_IMG = 512
NS = 4  # row strips of 128

MAGIC_A = 8388607.5
MAGIC_B = 8388608.0
TG22 = 0.4142135623730951
TG67 = 2.414213562373095

# shift-matrix stack indices
I_T_TOP, I_T_MID, I_T_BOT = 0, 1, 2
I_D_TOP, I_D_MID, I_D_BOT = 3, 4, 5
I_V, I_H_TOP, I_H_BOT, I_H_TOP_D = 6, 7, 8, 9
I_N, I_S = 10, 11
N_MATS = 12


def build_shift_mats():
    m = np.zeros((N_MATS, 128, 128), np.float16)
    i = np.arange(128)
    # vertical (1,2,1) smooth: out[p] = in[p-1] + 2 in[p] + in[p+1]
    for t in (I_T_TOP, I_T_MID, I_T_BOT):
        m[t][i, i] = 2.0
        m[t][i[:-1], i[1:]] = 1.0
        m[t][i[1:], i[:-1]] = 1.0
    m[I_T_TOP][0, 0] = 3.0      # replicate pad at image top
    m[I_T_BOT][127, 127] = 3.0  # replicate pad at image bottom
    # vertical diff: out[p] = in[p+1] - in[p-1]
    for t in (I_D_TOP, I_D_MID, I_D_BOT):
        m[t][i[1:], i[:-1]] = 1.0
        m[t][i[:-1], i[1:]] = -1.0
    m[I_D_TOP][0, 0] = -1.0       # out[0] = in[1] - in[0]
    m[I_D_BOT][127, 127] = 1.0    # out[127] = in[127] - in[126]
    m[I_V][i, i] = 1.0            # vertical (1,1,1) sum
    m[I_V][i[:-1], i[1:]] = 1.0
    m[I_V][i[1:], i[:-1]] = 1.0
    m[I_H_TOP][127, 0] = 1.0      # prev strip row 127 -> out row 0
    m[I_H_BOT][0, 127] = 1.0      # next strip row 0 -> out row 127
    m[I_H_TOP_D][127, 0] = -1.0   # diff halo: -in_prev[127]
    m[I_N][i[:-1], i[1:]] = 1.0   # out[p] = in[p-1]
    m[I_S][i[1:], i[:-1]] = 1.0   # out[p] = in[p+1]
    return m


def build_conv_banks(W, b):
    """Two fp16 lhsT banks [128, 4, 128] for the fused 2-pass conv.

    rhs tile partition p: hi rgb 16c+rr (c<3), hi edge 48+rr, lo rgb
    64+16c+rr, lo edge 112+rr; rr = row within the 16-row window.
    psum m = 16*oi + rr, och = 8*og + oi.
    Pass A: Wh on all rgb rows; edge rows carry (Wh_e, Wl_e).
    Pass B: Wl on all rgb rows; edge rows 0."""
    Wc = W.astype(np.float32).copy()
    Wc[:, 3] = Wc[:, 3] * np.float32(255.0)
    Wh = Wc.astype(np.float16)
    Wl = (Wc - Wh.astype(np.float32)).astype(np.float16)
    convA = np.zeros((128, 4, 128), np.float16)
    convB = np.zeros((128, 4, 128), np.float16)
    rr = np.arange(16)
    for og in range(4):
        for oi in range(8):
            oc = 8 * og + oi
            m = 16 * oi + rr
            for c in range(3):
                convA[16 * c + rr, og, m] = Wh[oc, c]
                convA[64 + 16 * c + rr, og, m] = Wh[oc, c]
                convB[16 * c + rr, og, m] = Wl[oc, c]
                convB[64 + 16 * c + rr, og, m] = Wl[oc, c]
            convA[48 + rr, og, m] = Wh[oc, 3]
            convA[112 + rr, og, m] = Wl[oc, 3]
    p = np.arange(128)
    brep = np.stack([b[8 * og + p // 16] for og in range(4)], axis=1)
    return convA, convB, brep.astype(np.float32)


def build_xfold(x):
    """Host-side conv operand packing: fp16 hi/lo split of x in the exact
    [s, 128, 8, 512] folded rhs layout (edge rows zero, filled on-chip)."""
    B = x.shape[0]
    xh = x.astype(np.float16)
    xl = (x - xh.astype(np.float32)).astype(np.float16)
    xfold = np.zeros((B, NS, 128, 8, 512), np.float16)
    # [B,3,512,512] -> [B,3,s,w,rr,j] -> [B,s,c,rr,w,j]
    for src, base in ((xh, 0), (xl, 64)):
        t = src.reshape(B, 3, NS, 8, 16, 512).transpose(0, 2, 1, 4, 3, 5)
        xfold[:, :, base:base + 48] = t.reshape(B, NS, 48, 8, 512)
    return np.ascontiguousarray(xfold)


_PROG_CACHE = {}


def build_program():
    import concourse.bacc as bacc
    import concourse.tile as tile
    import concourse.mybir as mybir
    from concourse.mybir import AluOpType as op, ActivationFunctionType as act
    from contextlib import ExitStack

    f32 = mybir.dt.float32
    f16 = mybir.dt.float16
    u8 = mybir.dt.uint8

    nc = bacc.Bacc("TRN2", target_bir_lowering=False, debug=False)
    x_d = nc.dram_tensor("x", [B_LOC, 3, H, W_IMG], f32, kind="ExternalInput").ap()
    xfold_d = nc.dram_tensor("xfold", [B_LOC, NS, 128, 8, 512], f16,
                             kind="ExternalInput").ap()
    mats_d = nc.dram_tensor("mats", [N_MATS, 128, 128], f16, kind="ExternalInput").ap()
    convA_d = nc.dram_tensor("convA", [128, 512], f16, kind="ExternalInput").ap()
    convB_d = nc.dram_tensor("convB", [128, 512], f16, kind="ExternalInput").ap()
    brep_d = nc.dram_tensor("brep", [128, 4], f32, kind="ExternalInput").ap()
    out_d = nc.dram_tensor("out", [B_LOC, NS, 4, 4, 128, 1024], f16,
                           kind="ExternalOutput").ap()
    e_d = nc.dram_tensor("escratch", [B_LOC, H, W_IMG], f16, kind="Internal").ap()

    with tile.TileContext(nc) as tc:
        with ExitStack() as ctx:
            ep = ctx.enter_context
            constp = ep(tc.tile_pool(name="const", bufs=1))
            rgbp = ep(tc.tile_pool(name="rgb", bufs=1))
            tmpp = ep(tc.tile_pool(name="tmp", bufs=1))
            gpadp = ep(tc.tile_pool(name="gpad", bufs=2))   # gpad + spdS share
            padp = ep(tc.tile_pool(name="pad", bufs=2))     # spad + npad share
            tplp = ep(tc.tile_pool(name="tpl", bufs=1))
            magp = ep(tc.tile_pool(name="mag", bufs=2))
            t16p = ep(tc.tile_pool(name="t16", bufs=2))
            selp = ep(tc.tile_pool(name="sel", bufs=2))
            mskp = ep(tc.tile_pool(name="msk", bufs=2))
            wkp = ep(tc.tile_pool(name="wk", bufs=2))
            curp = ep(tc.tile_pool(name="cur", bufs=3))
            xip = ep(tc.tile_pool(name="xi", bufs=3))
            ovp = ep(tc.tile_pool(name="ov", bufs=6))
            pvertp = ep(tc.tile_pool(name="pvert", bufs=2, space="PSUM"))
            pconvp = ep(tc.tile_pool(name="pconv", bufs=3, space="PSUM"))

            mats = constp.tile([128, N_MATS, 128], f16, tag="mats")
            nc.sync.dma_start(mats[:], mats_d.rearrange("m k n -> k m n"))
            convA = constp.tile([128, 4, 128], f16, tag="convA")
            nc.sync.dma_start(convA.rearrange("p g m -> p (g m)"), convA_d)
            convB = constp.tile([128, 4, 128], f16, tag="convB")
            nc.sync.dma_start(convB.rearrange("p g m -> p (g m)"), convB_d)
            brep = constp.tile([128, 4], f32, tag="brep")
            nc.sync.dma_start(brep[:], brep_d)

            def mat(idx):
                return mats[:, idx, :]

            # eviction engine rotation (GPSIMD has no PSUM port)
            EV = ["act", "vec", "act", "act", "vec", "act", "vec", "act"]

            st = [dict() for _ in range(B_LOC)]

            def p1_gray(bi):
                s_ = st[bi]
                tr = rgbp.tile([128, NS, 512], f32, tag="tr")
                tg = rgbp.tile([128, NS, 512], f32, tag="tg")
                tb = rgbp.tile([128, NS, 512], f32, tag="tb")
                nc.sync.dma_start(tr[:], x_d[bi, 0].rearrange("(s p) j -> p s j", s=NS))
                nc.sync.dma_start(tg[:], x_d[bi, 1].rearrange("(s p) j -> p s j", s=NS))
                nc.sync.dma_start(tb[:], x_d[bi, 2].rearrange("(s p) j -> p s j", s=NS))
                g1 = tmpp.tile([128, NS, 512], f32, tag="ta")
                nc.vector.tensor_scalar(g1[:], tr[:], 0.2989, None, op0=op.mult)
                g2 = tmpp.tile([128, NS, 512], f32, tag="tb2")
                nc.scalar.activation(g2[:], tg[:], act.Copy, bias=0.0, scale=0.587)
                g3 = tmpp.tile([128, NS, 512], f32, tag="tc")
                nc.gpsimd.tensor_tensor(g3[:], g1[:], g2[:], op=op.add)
                g4 = tmpp.tile([128, NS, 512], f32, tag="ta")
                nc.scalar.activation(g4[:], tb[:], act.Copy, bias=0.0, scale=0.114)
                gray = tmpp.tile([128, NS, 512], f32, tag="tb2")
                nc.vector.tensor_tensor(gray[:], g3[:], g4[:], op=op.add)
                # floor via magic round + fixup; the two rounding steps sit on
                # different engines so bacc cannot fuse them
                y1 = tmpp.tile([128, NS, 512], f32, tag="tc")
                nc.vector.tensor_scalar(y1[:], gray[:], MAGIC_A, None, op0=op.add)
                z1 = tmpp.tile([128, NS, 512], f32, tag="ta")
                nc.scalar.activation(z1[:], y1[:], act.Copy, bias=-MAGIC_B, scale=1.0)
                d1 = tmpp.tile([128, NS, 512], f32, tag="tc")
                nc.gpsimd.tensor_tensor(d1[:], gray[:], z1[:], op=op.subtract)
                gpad = gpadp.tile([128, NS, 514], f16, tag="gpad")
                nc.vector.scalar_tensor_tensor(
                    gpad[:, :, 1:513], d1[:], 1.0, z1[:], op0=op.is_ge, op1=op.add)
                nc.scalar.copy(gpad[:, :, 0:1], gpad[:, :, 1:2])
                nc.scalar.copy(gpad[:, :, 513:514], gpad[:, :, 512:513])
                s_["gpad"] = gpad

            def p23_sobel(bi):
                s_ = st[bi]
                gpad = s_["gpad"]
                u1 = t16p.tile([128, NS, 512], f16, tag="t16c")
                nc.vector.scalar_tensor_tensor(
                    u1[:], gpad[:, :, 1:513], 2.0, gpad[:, :, 0:512],
                    op0=op.mult, op1=op.add)
                tpl = tplp.tile([128, NS, 512], f16, tag="tpl")
                nc.gpsimd.tensor_tensor(tpl[:], u1[:], gpad[:, :, 2:514], op=op.add)
                spad = padp.tile([128, NS, 514], f16, tag="pad1")
                for s in range(NS):
                    ps = pvertp.tile([128, 512], f32, tag="pv")
                    tm = (I_T_TOP, I_T_MID, I_T_MID, I_T_BOT)[s]
                    nc.tensor.matmul(ps[:], mat(tm), gpad[:, s, 1:513],
                                     start=True, stop=False)
                    if s > 0:
                        nc.tensor.matmul(ps[:], mat(I_H_TOP), gpad[:, s - 1, 1:513],
                                         start=False, stop=(s == 3))
                    if s < 3:
                        nc.tensor.matmul(ps[:], mat(I_H_BOT), gpad[:, s + 1, 1:513],
                                         start=False, stop=True)
                    nc.scalar.copy(spad[:, s, 1:513], ps[:])
                nc.scalar.copy(spad[:, :, 0:1], spad[:, :, 1:2])
                nc.scalar.copy(spad[:, :, 513:514], spad[:, :, 512:513])
                gx = t16p.tile([128, NS, 512], f16, tag="t16a")
                nc.gpsimd.tensor_tensor(gx[:], spad[:, :, 2:514], spad[:, :, 0:512],
                                        op=op.subtract)
                gy = t16p.tile([128, NS, 512], f16, tag="t16b")
                for s in range(NS):
                    pg = pvertp.tile([128, 512], f32, tag="pv")
                    dm = (I_D_TOP, I_D_MID, I_D_MID, I_D_BOT)[s]
                    nc.tensor.matmul(pg[:], mat(dm), tpl[:, s, :],
                                     start=True, stop=False)
                    if s > 0:
                        nc.tensor.matmul(pg[:], mat(I_H_TOP_D), tpl[:, s - 1, :],
                                         start=False, stop=(s == 3))
                    if s < 3:
                        nc.tensor.matmul(pg[:], mat(I_H_BOT), tpl[:, s + 1, :],
                                         start=False, stop=True)
                    if s % 2 == 0:
                        nc.scalar.copy(gy[:, s, :], pg[:])
                    else:
                        nc.vector.tensor_copy(gy[:, s, :], pg[:])
                ax = selp.tile([128, NS, 512], f16, tag="sel1")
                nc.scalar.activation(ax[:], gx[:], act.Abs)
                ay = selp.tile([128, NS, 512], f16, tag="sel2")
                nc.scalar.activation(ay[:], gy[:], act.Abs)
                mag = magp.tile([128, NS, 514], f16, tag="mag")
                nc.gpsimd.tensor_tensor(mag[:, :, 1:513], ax[:], ay[:], op=op.add)
                nc.vector.memset(mag[:, :, 0:514:513], 0.0)
                hz = mskp.tile([128, NS, 512], u8, tag="hz")
                nc.vector.scalar_tensor_tensor(hz[:], ax[:], TG22, ay[:],
                                               op0=op.mult, op1=op.is_ge)
                vt = mskp.tile([128, NS, 512], u8, tag="vt")
                nc.vector.scalar_tensor_tensor(vt[:], ax[:], TG67, ay[:],
                                               op0=op.mult, op1=op.is_lt)
                sprod = t16p.tile([128, NS, 512], f16, tag="t16c")
                nc.gpsimd.tensor_tensor(sprod[:], gx[:], gy[:], op=op.mult)
                sn = mskp.tile([128, NS, 512], u8, tag="sn")
                nc.vector.tensor_scalar(sn[:], sprod[:], 0.0, None, op0=op.is_ge)
                s_.update(mag=mag, hz=hz, vt=vt, sn=sn)

            def p45_nms_hyst(bi):
                s_ = st[bi]
                mag, hz, vt, sn = s_["mag"], s_["hz"], s_["vt"], s_["sn"]
                npad = padp.tile([128, NS, 514], f16, tag="pad1")
                for s in range(NS):
                    pn = pvertp.tile([128, 512], f32, tag="pv")
                    nc.tensor.matmul(pn[:], mat(I_N), mag[:, s, 1:513],
                                     start=True, stop=(s == 0))
                    if s > 0:
                        nc.tensor.matmul(pn[:], mat(I_H_TOP), mag[:, s - 1, 1:513],
                                         start=False, stop=True)
                    if s % 2 == 0:
                        nc.scalar.copy(npad[:, s, 1:513], pn[:])
                    else:
                        nc.vector.tensor_copy(npad[:, s, 1:513], pn[:])
                nc.vector.memset(npad[:, :, 0:514:513], 0.0)
                spdS = gpadp.tile([128, NS, 514], f16, tag="gpad")
                for s in range(NS):
                    pss = pvertp.tile([128, 512], f32, tag="pv")
                    nc.tensor.matmul(pss[:], mat(I_S), mag[:, s, 1:513],
                                     start=True, stop=(s == 3))
                    if s < 3:
                        nc.tensor.matmul(pss[:], mat(I_H_BOT), mag[:, s + 1, 1:513],
                                         start=False, stop=True)
                    if s % 2 == 0:
                        nc.vector.tensor_copy(spdS[:, s, 1:513], pss[:])
                    else:
                        nc.scalar.copy(spdS[:, s, 1:513], pss[:])
                nc.vector.memset(spdS[:, :, 0:514:513], 0.0)
                # fwd = where(horiz, e, where(vert, n, where(ssn, nw, ne)))
                fwd = selp.tile([128, NS, 512], f16, tag="sel1")
                nc.scalar.copy(fwd[:], npad[:, :, 2:514])                      # ne
                nc.vector.copy_predicated(fwd[:], sn[:], npad[:, :, 0:512])    # nw
                nc.vector.copy_predicated(fwd[:], vt[:], npad[:, :, 1:513])    # n
                nc.vector.copy_predicated(fwd[:], hz[:], mag[:, :, 2:514])     # e
                bwd = selp.tile([128, NS, 512], f16, tag="sel2")
                nc.scalar.copy(bwd[:], spdS[:, :, 0:512])                      # sw
                nc.vector.copy_predicated(bwd[:], sn[:], spdS[:, :, 2:514])    # se
                nc.vector.copy_predicated(bwd[:], vt[:], spdS[:, :, 1:513])    # s
                nc.vector.copy_predicated(bwd[:], hz[:], mag[:, :, 0:512])     # w
                bigm = t16p.tile([128, NS, 512], f16, tag="t16a")
                nc.vector.scalar_tensor_tensor(bigm[:], fwd[:], 1.0, bwd[:],
                                               op0=op.add, op1=op.max)
                # keep = (mag > n1) & (mag >= n2) and thresholds, via integer
                # identity: cur = mag >= max(bigm, 151); wk = mag >= max(bigm, 51)
                cur = curp.tile([128, NS, 514], f16, tag="cpad")
                nc.vector.scalar_tensor_tensor(
                    cur[:, :, 1:513], bigm[:], 151.0, mag[:, :, 1:513],
                    op0=op.max, op1=op.is_le)
                nc.vector.memset(cur[:, :, 0:514:513], 0.0)
                wk = wkp.tile([128, NS, 512], f16, tag="wk")
                nc.vector.scalar_tensor_tensor(
                    wk[:], bigm[:], 51.0, mag[:, :, 1:513], op0=op.max, op1=op.is_le)

                # ---- hysteresis, 3 iterations -------------------------------
                for it in range(3):
                    h1 = selp.tile([128, NS, 512], f16, tag="sel1")
                    nc.gpsimd.tensor_tensor(h1[:], cur[:, :, 0:512],
                                            cur[:, :, 2:514], op=op.add)
                    ht = t16p.tile([128, NS, 512], f16, tag="t16c")
                    nc.vector.tensor_tensor(ht[:], h1[:], cur[:, :, 1:513], op=op.add)
                    cnew = curp.tile([128, NS, 514], f16, tag="cpad")
                    for s in range(NS):
                        pv = pvertp.tile([128, 512], f32, tag="pv")
                        nc.tensor.matmul(pv[:], mat(I_V), ht[:, s, :],
                                         start=True, stop=False)
                        if s > 0:
                            nc.tensor.matmul(pv[:], mat(I_H_TOP), ht[:, s - 1, :],
                                             start=False, stop=(s == 3))
                        if s < 3:
                            nc.tensor.matmul(pv[:], mat(I_H_BOT), ht[:, s + 1, :],
                                             start=False, stop=True)
                        nc.vector.scalar_tensor_tensor(
                            cnew[:, s, 1:513], pv[:], 0.0, wk[:, s, :],
                            op0=op.is_gt, op1=op.mult)
                    nc.vector.memset(cnew[:, :, 0:514:513], 0.0)
                    cur = cnew

                # edge channel round-trips through DRAM to reach the folded
                # conv layout (partition<->free exchange needs a DRAM AP)
                nc.sync.dma_start(e_d[bi].rearrange("(s p) j -> p s j", s=NS),
                                  cur[:, :, 1:513])

            def p6_conv(bi):
                for s in range(NS):
                    xi = xip.tile([128, 8, 512], f16, tag="xi")
                    nc.sync.dma_start(xi[:], xfold_d[bi, s])
                    nc.sync.dma_start(
                        xi[48:64],
                        e_d[bi, 128 * s:128 * s + 128, :]
                            .rearrange("(w r) j -> r w j", w=8))
                    nc.sync.dma_start(
                        xi[112:128],
                        e_d[bi, 128 * s:128 * s + 128, :]
                            .rearrange("(w r) j -> r w j", w=8))
                    for og in range(4):
                        for vg in range(2):  # window groups (v0,v1), (v2,v3)
                            pcs = []
                            for v in (2 * vg, 2 * vg + 1):
                                pc = pconvp.tile([128, 1024], f32, tag="pc")
                                nc.tensor.matmul(pc[:, 0:512], convA[:, og, :],
                                                 xi[:, 2 * v, :],
                                                 start=True, stop=False)
                                nc.tensor.matmul(pc[:, 512:1024], convA[:, og, :],
                                                 xi[:, 2 * v + 1, :],
                                                 start=True, stop=False)
                                pcs.append(pc)
                            for i, v in enumerate((2 * vg, 2 * vg + 1)):
                                pc = pcs[i]
                                nc.tensor.matmul(pc[:, 0:512], convB[:, og, :],
                                                 xi[:, 2 * v, :],
                                                 start=False, stop=True)
                                nc.tensor.matmul(pc[:, 512:1024], convB[:, og, :],
                                                 xi[:, 2 * v + 1, :],
                                                 start=False, stop=True)
                            for i, v in enumerate((2 * vg, 2 * vg + 1)):
                                pc = pcs[i]
                                ov = ovp.tile([128, 1024], f16, tag="ov")
                                eng = EV[(2 * og + s + v) % 8]
                                if eng == "act":
                                    nc.scalar.activation(
                                        ov[:], pc[:], act.Relu,
                                        bias=brep[:, og:og + 1], scale=1.0)
                                    nc.scalar.dma_start(out_d[bi, s, og, v], ov[:])
                                else:
                                    nc.vector.tensor_scalar(
                                        ov[:], pc[:], brep[:, og:og + 1], 0.0,
                                        op0=op.add, op1=op.max)
                                    nc.sync.dma_start(out_d[bi, s, og, v], ov[:])

            # staggered two-image software pipeline
            p1_gray(0)
            p23_sobel(0)
            p1_gray(1)
            p45_nms_hyst(0)
            p23_sobel(1)
            p6_conv(0)
            p45_nms_hyst(1)
            p6_conv(1)
    nc.compile()
    return nc


def _get_program():
    if "nc" not in _PROG_CACHE:
        _PROG_CACHE["nc"] = build_program()
    return _PROG_CACHE["nc"]


def build_in_maps(x, W, b):
    x = np.ascontiguousarray(np.asarray(x, dtype=np.float32))
    W = np.asarray(W, dtype=np.float32)
    b = np.asarray(b, dtype=np.float32)
    mats = build_shift_mats()
    cA, cB, brep = build_conv_banks(W, b)
    cA = np.ascontiguousarray(cA.reshape(128, 512))
    cB = np.ascontiguousarray(cB.reshape(128, 512))
    xfold = build_xfold(x)
    in_maps = []
    for core in range(N_CORES):
        lo, hi = B_LOC * core, B_LOC * (core + 1)
        in_maps.append({"x": np.ascontiguousarray(x[lo:hi]),
                        "xfold": np.ascontiguousarray(xfold[lo:hi]),
                        "mats": mats, "convA": cA, "convB": cB, "brep": brep})
    return in_maps


def untangle(raw):
    """[B_LOC, s, og, v, 128, 1024] f16 -> [B_LOC, 32, 512, 512] f32."""
    r = raw.reshape(B_LOC, NS, 4, 4, 8, 16, 2, 512)
    r = r.transpose(0, 2, 4, 1, 3, 6, 5, 7)  # bi, og, oi, s, v, v2, rr, j
    return r.reshape(B_LOC, 32, H, W_IMG).astype(np.float32)


def kernel(x: np.ndarray, W: np.ndarray, b: np.ndarray) -> np.ndarray:
    from concourse.bass_utils import run_bass_kernel_spmd

    nc = _get_program()
    in_maps = build_in_maps(x, W, b)
    res = run_bass_kernel_spmd(nc, in_maps, core_ids=list(range(N_CORES)))
    return np.concatenate([untangle(r["out"]) for r in res.results], axis=0)


# revision 25
# speedup vs baseline: 1.7800x; 1.0864x over previous
"""Trainium2 Bass kernel for grayscale+Canny+1x1-conv (nn_BFA_3015067042007).

Data-parallel over batch: 16 images -> 8 cores x 2 images.

Design (v5):
  - Whole-image tiles [128, 4, 514]: the 4 row-strips of an image are packed
    along the free dim, so every elementwise Canny op covers the full image
    in ONE instruction (amortizes per-instruction overhead 4x). Stencil
    matmuls run per strip on free-dim slices of the big tiles; strip-halo
    rows enter via 1-row shift matrices accumulated in PSUM.
  - Software-pipelined emission: the two images' phases are emitted
    staggered (P1-P3 of img1 between the NMS/hysteresis and conv of img0)
    so every engine queue always holds ready work from the other image.
  - Fused 2-pass fp16 conv: rhs tile per strip is [128, 8, 512] with
    pseudo-channels [Rh Gh Bh E | Rl Gl Bl E'] (hi/lo fp16 split of x along
    K). Pass A lhsT carries Wh on all rgb rows and (Wh_e, Wl_e) on the two
    edge-row groups; pass B carries Wl on rgb rows. Sum = (Wh+Wl)(xh+xl)
    + W_e*e with no dropped cross terms -> fp32-level accuracy with 64
    matmuls/strip (vs 96 for the 3-pass hi/lo scheme).
  - The hi/lo split of x is prepared host-side (operand packing, like the
    lhsT banks) in the exact [128, 8*512] folded layout, so each strip's
    conv rhs is ONE fully-contiguous 1MB DMA (8KB descriptors).
  - Edge channel rows round-trip through a DRAM scratch to reach the folded
    layout (partition<->free exchange requires a DRAM AP).
  - fp16 output (rel err ~5e-4 << 2e-2 tolerance) in an eviction-native
    DRAM layout [B, s, og, v, 128, 1024] -> every output store is one fully
    contiguous 256KB DMA (2KB descriptors); the host untangles + upcasts.
    Halves HBM write traffic AND keeps HWDGE descriptor generation (the v3
    bottleneck: ~6ns/descriptor, serialized on one queue) off the critical
    path. ACT-eviction stores issue on the scalar HWDGE ring (hidden under
    ACT ALU time), DVE-eviction stores on the sync ring.
  - strong/weak thresholds fused with NMS via integer identity:
    cur = (mag >= max(bigm, 151)), wk = (mag >= max(bigm, 51)) -- one
    scalar_tensor_tensor each instead of an is_ge tensor_tensor + two stt.
"""

import numpy as np

B_FULL = 16
N_CORES = 8
B_LOC = B_FULL // N_CORES
H = 512
W_IMG = 512
NS = 4  # row strips of 128

MAGIC_A = 8388607.5
MAGIC_B = 8388608.0
TG22 = 0.4142135623730951
TG67 = 2.414213562373095

# shift-matrix stack indices
I_T_TOP, I_T_MID, I_T_BOT = 0, 1, 2
I_D_TOP, I_D_MID, I_D_BOT = 3, 4, 5
I_V, I_H_TOP, I_H_BOT, I_H_TOP_D = 6, 7, 8, 9
I_N, I_S = 10, 11
N_MATS = 12


def build_shift_mats():
    m = np.zeros((N_MATS, 128, 128), np.float16)
    i = np.arange(128)
    # vertical (1,2,1) smooth: out[p] = in[p-1] + 2 in[p] + in[p+1]
    for t in (I_T_TOP, I_T_MID, I_T_BOT):
        m[t][i, i] = 2.0
        m[t][i[:-1], i[1:]] = 1.0
        m[t][i[1:], i[:-1]] = 1.0
    m[I_T_TOP][0, 0] = 3.0      # replicate pad at image top
    m[I_T_BOT][127, 127] = 3.0  # replicate pad at image bottom
    # vertical diff: out[p] = in[p+1] - in[p-1]
    for t in (I_D_TOP, I_D_MID, I_D_BOT):
        m[t][i[1:], i[:-1]] = 1.0
        m[t][i[:-1], i[1:]] = -1.0
    m[I_D_TOP][0, 0] = -1.0       # out[0] = in[1] - in[0]
    m[I_D_BOT][127, 127] = 1.0    # out[127] = in[127] - in[126]
    m[I_V][i, i] = 1.0            # vertical (1,1,1) sum
    m[I_V][i[:-1], i[1:]] = 1.0
    m[I_V][i[1:], i[:-1]] = 1.0
    m[I_H_TOP][127, 0] = 1.0      # prev strip row 127 -> out row 0
    m[I_H_BOT][0, 127] = 1.0      # next strip row 0 -> out row 127
    m[I_H_TOP_D][127, 0] = -1.0   # diff halo: -in_prev[127]
    m[I_N][i[:-1], i[1:]] = 1.0   # out[p] = in[p-1]
    m[I_S][i[1:], i[:-1]] = 1.0   # out[p] = in[p+1]
    return m


def build_conv_banks(W, b):
    """Two fp16 lhsT banks [128, 4, 128] for the fused 2-pass conv.

    rhs tile partition p: hi rgb 16c+rr (c<3), hi edge 48+rr, lo rgb
    64+16c+rr, lo edge 112+rr; rr = row within the 16-row window.
    psum m = 16*oi + rr, och = 8*og + oi.
    Pass A: Wh on all rgb rows; edge rows carry (Wh_e, Wl_e).
    Pass B: Wl on all rgb rows; edge rows 0."""
    Wc = W.astype(np.float32).copy()
    Wc[:, 3] = Wc[:, 3] * np.float32(255.0)
    Wh = Wc.astype(np.float16)
    Wl = (Wc - Wh.astype(np.float32)).astype(np.float16)
    convA = np.zeros((128, 4, 128), np.float16)
    convB = np.zeros((128, 4, 128), np.float16)
    rr = np.arange(16)
    for og in range(4):
        for oi in range(8):
            oc = 8 * og + oi
            m = 16 * oi + rr
            for c in range(3):
                convA[16 * c + rr, og, m] = Wh[oc, c]
                convA[64 + 16 * c + rr, og, m] = Wh[oc, c]
                convB[16 * c + rr, og, m] = Wl[oc, c]
                convB[64 + 16 * c + rr, og, m] = Wl[oc, c]
            convA[48 + rr, og, m] = Wh[oc, 3]
            convA[112 + rr, og, m] = Wl[oc, 3]
    p = np.arange(128)
    brep = np.stack([b[8 * og + p // 16] for og in range(4)], axis=1)
    return convA, convB, brep.astype(np.float32)


def build_xfold(x):
    """Host-side conv operand packing: fp16 hi/lo split of x in the exact
    [s, 128, 8, 512] folded rhs layout (edge rows zero, filled on-chip)."""
    B = x.shape[0]
    xh = x.astype(np.float16)
    xl = (x - xh.astype(np.float32)).astype(np.float16)
    xfold = np.zeros((B, NS, 128, 8, 512), np.float16)
    # [B,3,512,512] -> [B,3,s,w,rr,j] -> [B,s,c,rr,w,j]
    for src, base in ((xh, 0), (xl, 64)):
        t = src.reshape(B, 3, NS, 8, 16, 512).transpose(0, 2, 1, 4, 3, 5)
        xfold[:, :, base:base + 48] = t.reshape(B, NS, 48, 8, 512)
    return np.ascontiguousarray(xfold)


_PROG_CACHE = {}


def build_program():
    import concourse.bacc as bacc
    import concourse.tile as tile
    import concourse.mybir as mybir
    from concourse.mybir import AluOpType as op, ActivationFunctionType as act
    from contextlib import ExitStack

    f32 = mybir.dt.float32
    f16 = mybir.dt.float16
    u8 = mybir.dt.uint8

    nc = bacc.Bacc("TRN2", target_bir_lowering=False, debug=False)
    x_d = nc.dram_tensor("x", [B_LOC, 3, H, W_IMG], f32, kind="ExternalInput").ap()
    xfold_d = nc.dram_tensor("xfold", [B_LOC, NS, 128, 8, 512], f16,
                             kind="ExternalInput").ap()
    mats_d = nc.dram_tensor("mats", [N_MATS, 128, 128], f16, kind="ExternalInput").ap()
    convA_d = nc.dram_tensor("convA", [128, 512], f16, kind="ExternalInput").ap()
    convB_d = nc.dram_tensor("convB", [128, 512], f16, kind="ExternalInput").ap()
    brep_d = nc.dram_tensor("brep", [128, 4], f32, kind="ExternalInput").ap()
    out_d = nc.dram_tensor("out", [B_LOC, NS, 4, 4, 128, 1024], f16,
                           kind="ExternalOutput").ap()
    e_d = nc.dram_tensor("escratch", [B_LOC, H, W_IMG], f16, kind="Internal").ap()

    with tile.TileContext(nc) as tc:
        with ExitStack() as ctx:
            ep = ctx.enter_context
            constp = ep(tc.tile_pool(name="const", bufs=1))
            rgbp = ep(tc.tile_pool(name="rgb", bufs=1))
            tmpp = ep(tc.tile_pool(name="tmp", bufs=1))
            gpadp = ep(tc.tile_pool(name="gpad", bufs=2))   # gpad + spdS share
            padp = ep(tc.tile_pool(name="pad", bufs=2))     # spad + npad share
            tplp = ep(tc.tile_pool(name="tpl", bufs=1))
            magp = ep(tc.tile_pool(name="mag", bufs=2))
            t16p = ep(tc.tile_pool(name="t16", bufs=2))
            selp = ep(tc.tile_pool(name="sel", bufs=3))
            mskp = ep(tc.tile_pool(name="msk", bufs=2))
            wkp = ep(tc.tile_pool(name="wk", bufs=2))
            curp = ep(tc.tile_pool(name="cur", bufs=3))
            xip = ep(tc.tile_pool(name="xi", bufs=3))
            ovp = ep(tc.tile_pool(name="ov", bufs=6))
            pvertp = ep(tc.tile_pool(name="pvert", bufs=2, space="PSUM"))
            pconvp = ep(tc.tile_pool(name="pconv", bufs=2, space="PSUM"))

            mats = constp.tile([128, N_MATS, 128], f16, tag="mats")
            nc.sync.dma_start(mats[:], mats_d.rearrange("m k n -> k m n"))
            convA = constp.tile([128, 4, 128], f16, tag="convA")
            nc.sync.dma_start(convA.rearrange("p g m -> p (g m)"), convA_d)
            convB = constp.tile([128, 4, 128], f16, tag="convB")
            nc.sync.dma_start(convB.rearrange("p g m -> p (g m)"), convB_d)
            brep = constp.tile([128, 4], f32, tag="brep")
            nc.sync.dma_start(brep[:], brep_d)

            def mat(idx):
                return mats[:, idx, :]

            # eviction engine rotation (GPSIMD has no PSUM port)
            EV = ["act", "vec", "act", "act", "vec", "act", "vec", "act"]

            st = [dict() for _ in range(B_LOC)]

            def p1_gray(bi):
                s_ = st[bi]
                tr = rgbp.tile([128, NS, 512], f32, tag="tr")
                tg = rgbp.tile([128, NS, 512], f32, tag="tg")
                tb = rgbp.tile([128, NS, 512], f32, tag="tb")
                nc.sync.dma_start(tr[:], x_d[bi, 0].rearrange("(s p) j -> p s j", s=NS))
                nc.sync.dma_start(tg[:], x_d[bi, 1].rearrange("(s p) j -> p s j", s=NS))
                nc.sync.dma_start(tb[:], x_d[bi, 2].rearrange("(s p) j -> p s j", s=NS))
                g1 = tmpp.tile([128, NS, 512], f32, tag="ta")
                nc.vector.tensor_scalar(g1[:], tr[:], 0.2989, None, op0=op.mult)
                g2 = tmpp.tile([128, NS, 512], f32, tag="tb2")
                nc.scalar.activation(g2[:], tg[:], act.Copy, bias=0.0, scale=0.587)
                g3 = tmpp.tile([128, NS, 512], f32, tag="tc")
                nc.gpsimd.tensor_tensor(g3[:], g1[:], g2[:], op=op.add)
                g4 = tmpp.tile([128, NS, 512], f32, tag="ta")
                nc.scalar.activation(g4[:], tb[:], act.Copy, bias=0.0, scale=0.114)
                gray = tmpp.tile([128, NS, 512], f32, tag="tb2")
                nc.vector.tensor_tensor(gray[:], g3[:], g4[:], op=op.add)
                # floor via magic round + fixup; the two rounding steps sit on
                # different engines so bacc cannot fuse them
                y1 = tmpp.tile([128, NS, 512], f32, tag="tc")
                nc.vector.tensor_scalar(y1[:], gray[:], MAGIC_A, None, op0=op.add)
                z1 = tmpp.tile([128, NS, 512], f32, tag="ta")
                nc.scalar.activation(z1[:], y1[:], act.Copy, bias=-MAGIC_B, scale=1.0)
                d1 = tmpp.tile([128, NS, 512], f32, tag="tc")
                nc.gpsimd.tensor_tensor(d1[:], gray[:], z1[:], op=op.subtract)
                gpad = gpadp.tile([128, NS, 514], f16, tag="gpad")
                nc.vector.scalar_tensor_tensor(
                    gpad[:, :, 1:513], d1[:], 1.0, z1[:], op0=op.is_ge, op1=op.add)
                nc.scalar.copy(gpad[:, :, 0:1], gpad[:, :, 1:2])
                nc.scalar.copy(gpad[:, :, 513:514], gpad[:, :, 512:513])
                s_["gpad"] = gpad

            def p2_smooth(bi):
                s_ = st[bi]
                gpad = s_["gpad"]
                u1 = t16p.tile([128, NS, 512], f16, tag="t16c")
                nc.vector.scalar_tensor_tensor(
                    u1[:], gpad[:, :, 1:513], 2.0, gpad[:, :, 0:512],
                    op0=op.mult, op1=op.add)
                tpl = tplp.tile([128, NS, 512], f16, tag="tpl")
                nc.gpsimd.tensor_tensor(tpl[:], u1[:], gpad[:, :, 2:514], op=op.add)
                spad = padp.tile([128, NS, 514], f16, tag="pad1")
                for sp in range(2):  # strip pairs in 2-bank psum tiles
                    ps = pvertp.tile([128, 1024], f32, tag="pv")
                    for k, s in enumerate((2 * sp, 2 * sp + 1)):
                        seg = ps[:, 512 * k:512 * k + 512]
                        tm = (I_T_TOP, I_T_MID, I_T_MID, I_T_BOT)[s]
                        nc.tensor.matmul(seg, mat(tm), gpad[:, s, 1:513],
                                         start=True, stop=False)
                        if s > 0:
                            nc.tensor.matmul(seg, mat(I_H_TOP), gpad[:, s - 1, 1:513],
                                             start=False, stop=(s == 3))
                        if s < 3:
                            nc.tensor.matmul(seg, mat(I_H_BOT), gpad[:, s + 1, 1:513],
                                             start=False, stop=True)
                    nc.scalar.copy(spad[:, 2 * sp:2 * sp + 2, 1:513], ps[:])
                nc.scalar.copy(spad[:, :, 0:1], spad[:, :, 1:2])
                nc.scalar.copy(spad[:, :, 513:514], spad[:, :, 512:513])
                gx = t16p.tile([128, NS, 512], f16, tag="t16a")
                nc.gpsimd.tensor_tensor(gx[:], spad[:, :, 2:514], spad[:, :, 0:512],
                                        op=op.subtract)
                ax = selp.tile([128, NS, 512], f16, tag="sel1")
                nc.scalar.activation(ax[:], gx[:], act.Abs)
                s_.update(tpl=tpl, gx=gx, ax=ax)

            def p3_grad(bi):
                s_ = st[bi]
                tpl, gx, ax = s_["tpl"], s_["gx"], s_["ax"]
                gy = t16p.tile([128, NS, 512], f16, tag="t16b")
                for sp in range(2):
                    pg = pvertp.tile([128, 1024], f32, tag="pv")
                    for k, s in enumerate((2 * sp, 2 * sp + 1)):
                        seg = pg[:, 512 * k:512 * k + 512]
                        dm = (I_D_TOP, I_D_MID, I_D_MID, I_D_BOT)[s]
                        nc.tensor.matmul(seg, mat(dm), tpl[:, s, :],
                                         start=True, stop=False)
                        if s > 0:
                            nc.tensor.matmul(seg, mat(I_H_TOP_D), tpl[:, s - 1, :],
                                             start=False, stop=(s == 3))
                        if s < 3:
                            nc.tensor.matmul(seg, mat(I_H_BOT), tpl[:, s + 1, :],
                                             start=False, stop=True)
                    if sp == 0:
                        nc.scalar.copy(gy[:, 0:2, :], pg[:])
                    else:
                        nc.vector.tensor_copy(gy[:, 2:4, :], pg[:])
                ay = selp.tile([128, NS, 512], f16, tag="sel2")
                nc.scalar.activation(ay[:], gy[:], act.Abs)
                mag = magp.tile([128, NS, 514], f16, tag="mag")
                nc.gpsimd.tensor_tensor(mag[:, :, 1:513], ax[:], ay[:], op=op.add)
                nc.vector.memset(mag[:, :, 0:514:513], 0.0)
                s_.update(gy=gy, ay=ay, mag=mag)

            def p3_masks(bi):
                s_ = st[bi]
                gx, gy, ax, ay = s_["gx"], s_["gy"], s_["ax"], s_["ay"]
                hz = mskp.tile([128, NS, 512], u8, tag="hz")
                nc.vector.scalar_tensor_tensor(hz[:], ax[:], TG22, ay[:],
                                               op0=op.mult, op1=op.is_ge)
                vt = mskp.tile([128, NS, 512], u8, tag="vt")
                nc.vector.scalar_tensor_tensor(vt[:], ax[:], TG67, ay[:],
                                               op0=op.mult, op1=op.is_lt)
                sprod = t16p.tile([128, NS, 512], f16, tag="t16c")
                nc.gpsimd.tensor_tensor(sprod[:], gx[:], gy[:], op=op.mult)
                sn = mskp.tile([128, NS, 512], u8, tag="sn")
                nc.vector.tensor_scalar(sn[:], sprod[:], 0.0, None, op0=op.is_ge)
                s_.update(hz=hz, vt=vt, sn=sn)

            def p4_nms(bi):
                s_ = st[bi]
                mag, hz, vt, sn = s_["mag"], s_["hz"], s_["vt"], s_["sn"]
                npad = padp.tile([128, NS, 514], f16, tag="pad1")
                for sp in range(2):
                    pn = pvertp.tile([128, 1024], f32, tag="pv")
                    for k, s in enumerate((2 * sp, 2 * sp + 1)):
                        seg = pn[:, 512 * k:512 * k + 512]
                        nc.tensor.matmul(seg, mat(I_N), mag[:, s, 1:513],
                                         start=True, stop=(s == 0))
                        if s > 0:
                            nc.tensor.matmul(seg, mat(I_H_TOP), mag[:, s - 1, 1:513],
                                             start=False, stop=True)
                    if sp == 0:
                        nc.scalar.copy(npad[:, 0:2, 1:513], pn[:])
                    else:
                        nc.vector.tensor_copy(npad[:, 2:4, 1:513], pn[:])
                nc.vector.memset(npad[:, :, 0:514:513], 0.0)
                spdS = gpadp.tile([128, NS, 514], f16, tag="gpad")
                for sp in range(2):
                    pss = pvertp.tile([128, 1024], f32, tag="pv")
                    for k, s in enumerate((2 * sp, 2 * sp + 1)):
                        seg = pss[:, 512 * k:512 * k + 512]
                        nc.tensor.matmul(seg, mat(I_S), mag[:, s, 1:513],
                                         start=True, stop=(s == 3))
                        if s < 3:
                            nc.tensor.matmul(seg, mat(I_H_BOT), mag[:, s + 1, 1:513],
                                             start=False, stop=True)
                    if sp == 0:
                        nc.vector.tensor_copy(spdS[:, 0:2, 1:513], pss[:])
                    else:
                        nc.scalar.copy(spdS[:, 2:4, 1:513], pss[:])
                nc.vector.memset(spdS[:, :, 0:514:513], 0.0)
                # fwd = where(horiz, e, where(vert, n, where(ssn, nw, ne)))
                fwd = selp.tile([128, NS, 512], f16, tag="sel1")
                nc.scalar.copy(fwd[:], npad[:, :, 2:514])                      # ne
                nc.vector.copy_predicated(fwd[:], sn[:], npad[:, :, 0:512])    # nw
                nc.vector.copy_predicated(fwd[:], vt[:], npad[:, :, 1:513])    # n
                nc.vector.copy_predicated(fwd[:], hz[:], mag[:, :, 2:514])     # e
                bwd = selp.tile([128, NS, 512], f16, tag="sel2")
                nc.scalar.copy(bwd[:], spdS[:, :, 0:512])                      # sw
                nc.vector.copy_predicated(bwd[:], sn[:], spdS[:, :, 2:514])    # se
                nc.vector.copy_predicated(bwd[:], vt[:], spdS[:, :, 1:513])    # s
                nc.vector.copy_predicated(bwd[:], hz[:], mag[:, :, 0:512])     # w
                bigm = t16p.tile([128, NS, 512], f16, tag="t16a")
                nc.vector.scalar_tensor_tensor(bigm[:], fwd[:], 1.0, bwd[:],
                                               op0=op.add, op1=op.max)
                # keep = (mag > n1) & (mag >= n2) and thresholds, via integer
                # identity: cur = mag >= max(bigm, 151); wk = mag >= max(bigm, 51)
                cur = curp.tile([128, NS, 514], f16, tag="cpad")
                nc.vector.scalar_tensor_tensor(
                    cur[:, :, 1:513], bigm[:], 151.0, mag[:, :, 1:513],
                    op0=op.max, op1=op.is_le)
                nc.vector.memset(cur[:, :, 0:514:513], 0.0)
                wk = wkp.tile([128, NS, 512], f16, tag="wk")
                nc.vector.scalar_tensor_tensor(
                    wk[:], bigm[:], 51.0, mag[:, :, 1:513], op0=op.max, op1=op.is_le)
                s_.update(cur=cur, wk=wk)

            def hyst_iter(bi):
                s_ = st[bi]
                cur, wk = s_["cur"], s_["wk"]
                h1 = selp.tile([128, NS, 512], f16, tag="sel1")
                nc.vector.tensor_tensor(h1[:], cur[:, :, 0:512],
                                        cur[:, :, 2:514], op=op.add)
                ht = t16p.tile([128, NS, 512], f16, tag="t16c")
                nc.vector.tensor_tensor(ht[:], h1[:], cur[:, :, 1:513], op=op.add)
                cnew = curp.tile([128, NS, 514], f16, tag="cpad")
                for sp in range(2):
                    pv = pvertp.tile([128, 1024], f32, tag="pv")
                    for k, s in enumerate((2 * sp, 2 * sp + 1)):
                        seg = pv[:, 512 * k:512 * k + 512]
                        nc.tensor.matmul(seg, mat(I_V), ht[:, s, :],
                                         start=True, stop=False)
                        if s > 0:
                            nc.tensor.matmul(seg, mat(I_H_TOP), ht[:, s - 1, :],
                                             start=False, stop=(s == 3))
                        if s < 3:
                            nc.tensor.matmul(seg, mat(I_H_BOT), ht[:, s + 1, :],
                                             start=False, stop=True)
                    nc.vector.scalar_tensor_tensor(
                        cnew[:, 2 * sp:2 * sp + 2, 1:513], pv[:],
                        0.0, wk[:, 2 * sp:2 * sp + 2, :],
                        op0=op.is_gt, op1=op.mult)
                nc.vector.memset(cnew[:, :, 0:514:513], 0.0)
                s_["cur"] = cnew

            def edge_out(bi):
                # edge channel round-trips through DRAM to reach the folded
                # conv layout (partition<->free exchange needs a DRAM AP)
                nc.sync.dma_start(e_d[bi].rearrange("(s p) j -> p s j", s=NS),
                                  st[bi]["cur"][:, :, 1:513])

            def p6_conv(bi, strips):
                for s in strips:
                    xi = xip.tile([128, 8, 512], f16, tag="xi")
                    nc.sync.dma_start(xi[:], xfold_d[bi, s])
                    nc.sync.dma_start(
                        xi[48:64],
                        e_d[bi, 128 * s:128 * s + 128, :]
                            .rearrange("(w r) j -> r w j", w=8))
                    nc.sync.dma_start(
                        xi[112:128],
                        e_d[bi, 128 * s:128 * s + 128, :]
                            .rearrange("(w r) j -> r w j", w=8))
                    for og in range(4):
                        for vg in range(2):  # window groups (v0,v1), (v2,v3)
                            pcs = []
                            for v in (2 * vg, 2 * vg + 1):
                                pc = pconvp.tile([128, 1024], f32, tag="pc")
                                nc.tensor.matmul(pc[:, 0:512], convA[:, og, :],
                                                 xi[:, 2 * v, :],
                                                 start=True, stop=False)
                                nc.tensor.matmul(pc[:, 512:1024], convA[:, og, :],
                                                 xi[:, 2 * v + 1, :],
                                                 start=True, stop=False)
                                pcs.append(pc)
                            for i, v in enumerate((2 * vg, 2 * vg + 1)):
                                pc = pcs[i]
                                nc.tensor.matmul(pc[:, 0:512], convB[:, og, :],
                                                 xi[:, 2 * v, :],
                                                 start=False, stop=True)
                                nc.tensor.matmul(pc[:, 512:1024], convB[:, og, :],
                                                 xi[:, 2 * v + 1, :],
                                                 start=False, stop=True)
                            for i, v in enumerate((2 * vg, 2 * vg + 1)):
                                pc = pcs[i]
                                ov = ovp.tile([128, 1024], f16, tag="ov")
                                eng = EV[(2 * og + s + v) % 8]
                                if eng == "act":
                                    nc.scalar.activation(
                                        ov[:], pc[:], act.Relu,
                                        bias=brep[:, og:og + 1], scale=1.0)
                                    nc.scalar.dma_start(out_d[bi, s, og, v], ov[:])
                                else:
                                    nc.vector.tensor_scalar(
                                        ov[:], pc[:], brep[:, og:og + 1], 0.0,
                                        op0=op.add, op1=op.max)
                                    nc.sync.dma_start(out_d[bi, s, og, v], ov[:])

            # staggered two-image software pipeline, interleaved at sub-phase
            # granularity so no engine queue head-of-line blocks on a long
            # dependency chain while ready work exists
            p1_gray(0)
            p2_smooth(0)
            p3_grad(0)
            p3_masks(0)
            p1_gray(1)
            p4_nms(0)
            p2_smooth(1)
            hyst_iter(0)
            p3_grad(1)
            hyst_iter(0)
            p3_masks(1)
            hyst_iter(0)
            edge_out(0)
            p4_nms(1)
            p6_conv(0, (0, 1))
            hyst_iter(1)
            p6_conv(0, (2, 3))
            hyst_iter(1)
            hyst_iter(1)
            edge_out(1)
            p6_conv(1, (0, 1, 2, 3))
    nc.compile()
    return nc


def _get_program():
    if "nc" not in _PROG_CACHE:
        _PROG_CACHE["nc"] = build_program()
    return _PROG_CACHE["nc"]


def build_in_maps(x, W, b):
    x = np.ascontiguousarray(np.asarray(x, dtype=np.float32))
    W = np.asarray(W, dtype=np.float32)
    b = np.asarray(b, dtype=np.float32)
    mats = build_shift_mats()
    cA, cB, brep = build_conv_banks(W, b)
    cA = np.ascontiguousarray(cA.reshape(128, 512))
    cB = np.ascontiguousarray(cB.reshape(128, 512))
    xfold = build_xfold(x)
    in_maps = []
    for core in range(N_CORES):
        lo, hi = B_LOC * core, B_LOC * (core + 1)
        in_maps.append({"x": np.ascontiguousarray(x[lo:hi]),
                        "xfold": np.ascontiguousarray(xfold[lo:hi]),
                        "mats": mats, "convA": cA, "convB": cB, "brep": brep})
    return in_maps


def untangle(raw):
    """[B_LOC, s, og, v, 128, 1024] f16 -> [B_LOC, 32, 512, 512] f32."""
    r = raw.reshape(B_LOC, NS, 4, 4, 8, 16, 2, 512)
    r = r.transpose(0, 2, 4, 1, 3, 6, 5, 7)  # bi, og, oi, s, v, v2, rr, j
    return r.reshape(B_LOC, 32, H, W_IMG).astype(np.float32)


def kernel(x: np.ndarray, W: np.ndarray, b: np.ndarray) -> np.ndarray:
    from concourse.bass_utils import run_bass_kernel_spmd

    nc = _get_program()
    in_maps = build_in_maps(x, W, b)
    res = run_bass_kernel_spmd(nc, in_maps, core_ids=list(range(N_CORES)))
    return np.concatenate([untangle(r["out"]) for r in res.results], axis=0)
